# revision 1
# baseline (speedup 1.0000x reference)
"""Block-sparse (banded) attention kernel for Trainium2, 8 NeuronCores.

Sharding: data-parallel over batch (2) x tensor-parallel over heads
(16 heads -> 4 per core).  Each core computes its 4 heads' Q/K/V
projections, banded block attention (|r-c| <= 15 blocks, per-block
softmax), and a partial output projection; the host sums the 4 partial
outputs per batch element.

Self-contained: hardcodes all shapes; only needs the concourse tree that
the environment already puts on sys.path.
"""

import sys

for _p in ("/opt/trn_rl_repo",):
    if _p not in sys.path:
        sys.path.insert(0, _p)

from contextlib import ExitStack

import numpy as np

import concourse.bacc as bacc
import concourse.tile as tile
from concourse import bass_utils, mybir

F32 = mybir.dt.float32
F32R = mybir.dt.float32r
BF16 = mybir.dt.bfloat16
EXP = mybir.ActivationFunctionType.Exp

B, S, E = 2, 2048, 1024
H, HD, BLK = 16, 64, 64
NB = S // BLK  # 32 blocks
NCORES = 8
HPC = 4  # heads per core
F = HPC * HD  # 256 local features
BAND = 15
SCALE = HD ** -0.5

# per r8-slab (8 query blocks, q=512) column-block ranges, even-extended
T_SLABS = 4
QS = 512  # q extent per slab
LO = []
NP_T = []
for _t in range(T_SLABS):
    lo = max(0, 8 * _t - BAND)
    hi = min(NB - 1, 8 * _t + 7 + BAND)
    if (hi - lo + 1) % 2 == 1:
        if lo > 0:
            lo -= 1
        else:
            hi += 1
    LO.append(lo)
    NP_T.append((hi - lo + 1) // 2)
MAXP = max(NP_T)  # 16 pairs


def build_nc(debug=False):
    nc = bacc.Bacc("TRN2", target_bir_lowering=False, debug=False)

    xq_d = nc.dram_tensor("xqT", [E, S], F32R, kind="ExternalInput")
    xk_d = nc.dram_tensor("xkT", [E, S], F32R, kind="ExternalInput")
    xv_d = nc.dram_tensor("xvT", [E, S], F32R, kind="ExternalInput")
    wq_d = nc.dram_tensor("wqT", [E, F], F32R, kind="ExternalInput")
    wk_d = nc.dram_tensor("wkT", [E, F], F32R, kind="ExternalInput")
    wv_d = nc.dram_tensor("wvT", [E, F], F32R, kind="ExternalInput")
    wo_d = nc.dram_tensor("woT", [F, E], F32R, kind="ExternalInput")
    sel_d = nc.dram_tensor("selc", [128, MAXP * 32], F32R, kind="ExternalInput")
    bds_d = nc.dram_tensor("bdsel", [32, MAXP * 128], F32R, kind="ExternalInput")
    vm_d = nc.dram_tensor("vmask", [32, T_SLABS * QS], F32R, kind="ExternalInput")
    out_d = nc.dram_tensor("out", [S, E], F32, kind="ExternalOutput")
    if debug:
        qT_d = nc.dram_tensor("qT_dbg", [128, 2 * S], F32, kind="ExternalOutput")
        kT_d = nc.dram_tensor("kT_dbg", [128, 2 * S], F32, kind="ExternalOutput")
        vv_d = nc.dram_tensor("vv_dbg", [128, 16 * F], F32, kind="ExternalOutput")
        at_d = nc.dram_tensor("at_dbg", [F, S], F32, kind="ExternalOutput")

    with tile.TileContext(nc) as tc, ExitStack() as ctx, nc.allow_low_precision(
        reason="float32r pipeline; fp32 data format throughout"
    ):
        pers = ctx.enter_context(tc.tile_pool(name="pers", bufs=1))
        qT = pers.tile([128, 2 * S], F32R, tag="qT")
        kT = pers.tile([128, 2 * S], F32R, tag="kT")
        vv = pers.tile([128, 16 * F], F32R, tag="vv")
        wq = pers.tile([128, 8 * F], F32R, tag="wq")
        wk = pers.tile([128, 8 * F], F32R, tag="wk")
        wv = pers.tile([128, 8 * F], F32R, tag="wv")
        wo = pers.tile([64, 4 * E], F32R, tag="wo")
        selb = pers.tile([128, MAXP * 32], BF16, tag="selb")
        bds = pers.tile([32, MAXP * 128], F32R, tag="bds")
        vm = pers.tile([32, T_SLABS * QS], F32R, tag="vm")

        # k-projection weights first: phase 1 is on the critical path
        nc.sync.dma_start(
            wk[:].rearrange("p (c f) -> p c f", c=8),
            wk_d.ap().rearrange("(c p) f -> p c f", p=128),
        )
        # remaining weights/constants arrive via gpsimd (SWDGE) so they don't
        # queue ahead of the phase-1/2 x-tile loads on the sync ring
        nc.gpsimd.dma_start(
            wv[:].rearrange("p (c f) -> p c f", c=8),
            wv_d.ap().rearrange("(c p) f -> p c f", p=128),
        )
        nc.gpsimd.dma_start(
            wq[:].rearrange("p (c f) -> p c f", c=8),
            wq_d.ap().rearrange("(c p) f -> p c f", p=128),
        )
        nc.gpsimd.dma_start(
            wo[:].rearrange("p (c e) -> p c e", c=4),
            wo_d.ap().rearrange("(c p) e -> p c e", p=64),
        )
        nc.gpsimd.dma_start(selb[:], sel_d.ap())  # SWDGE casts f32 -> bf16
        nc.gpsimd.dma_start(bds[:], bds_d.ap())
        nc.gpsimd.dma_start(vm[:], vm_d.ap())

        # ---- phase 1: k projection (kT layout [f, s]) ----
        with tc.tile_pool(name="xk", bufs=2) as xkp, tc.tile_pool(
            name="psK", bufs=1, space="PSUM"
        ) as pskp:
            psK = pskp.tile([128, 4096], F32)
            for e in range(8):
                xt = xkp.tile([128, S], F32R, tag="xk")
                nc.sync.dma_start(xt[:], xk_d.ap()[e * 128 : (e + 1) * 128, :])
                for fold in range(2):
                    for sc in range(4):
                        nc.tensor.matmul(
                            psK[:, (fold * 4 + sc) * 512 : (fold * 4 + sc + 1) * 512],
                            wk[:, e * F + fold * 128 : e * F + fold * 128 + 128],
                            xt[:, sc * 512 : (sc + 1) * 512],
                            start=(e == 0),
                            stop=(e == 7),
                        )
            for fold in range(2):
                for sc in range(4):
                    nc.scalar.copy(
                        kT[:, fold * S + sc * 512 : fold * S + (sc + 1) * 512],
                        psK[:, (fold * 4 + sc) * 512 : (fold * 4 + sc + 1) * 512],
                    )

        # ---- phase 2: v projection (natural layout [s, f]) ----
        with tc.tile_pool(name="xv", bufs=3) as xvp, tc.tile_pool(
            name="psV", bufs=2, space="PSUM"
        ) as psvp:
            for sc in range(4):
                # one PSUM bank per sub-chunk: accumulation groups must not
                # interleave within a bank
                pvs = [
                    psvp.tile([128, 256], F32, name=f"pv{sub}", tag=f"psV{sub}")
                    for sub in range(4)
                ]
                for e in range(8):
                    xt = xvp.tile([128, 512], F32R, tag="xv")
                    nc.sync.dma_start(
                        xt[:],
                        xv_d.ap()[e * 128 : (e + 1) * 128, sc * 512 : (sc + 1) * 512],
                    )
                    for sub in range(4):
                        nc.tensor.matmul(
                            pvs[sub][:],
                            xt[:, sub * 128 : (sub + 1) * 128],
                            wv[:, e * F : (e + 1) * F],
                            start=(e == 0),
                            stop=(e == 7),
                        )
                for sub in range(4):
                    nc.scalar.copy(
                        vv[:, sc * 1024 + sub * 256 : sc * 1024 + (sub + 1) * 256],
                        pvs[sub][:],
                    )

        # ---- phase 3: q projection + attention + output projection ----
        xqp = ctx.enter_context(tc.tile_pool(name="xq", bufs=3))
        psSp = ctx.enter_context(tc.tile_pool(name="psS", bufs=6, space="PSUM"))
        
        flexp = ctx.enter_context(tc.tile_pool(name="flex", bufs=2, space="PSUM"))
        expp = ctx.enter_context(tc.tile_pool(name="expS", bufs=2))
        ptp = ctx.enter_context(tc.tile_pool(name="pt", bufs=4))
        rcpp = ctx.enter_context(tc.tile_pool(name="rcp", bufs=2))
        attp = ctx.enter_context(tc.tile_pool(name="att", bufs=8))
        outp = ctx.enter_context(tc.tile_pool(name="outsb", bufs=2))

        def unitA(h, t):
            npt = NP_T[t]
            lo = LO[t]
            fold = h // 2
            bp = 64 * (h % 2)  # partition base of this head's qT/kT rows
            expS = expp.tile([128, MAXP * QS], BF16, tag="expS")
            accs = psSp.tile([128, 512], F32, name="accs", tag="psS")
            for j in range(npt):
                c0 = lo + 2 * j
                ps = psSp.tile([128, 512], F32, name="ps", tag="psS")
                nc.tensor.matmul(
                    ps[:],
                    kT[bp : bp + 64, fold * S + c0 * 64 : fold * S + c0 * 64 + 128],
                    qT[bp : bp + 64, fold * S + t * QS : fold * S + (t + 1) * QS],
                    start=True,
                    stop=True,
                )
                nc.scalar.activation(
                    expS[:, j * QS : (j + 1) * QS], ps[:], EXP
                )
                nc.tensor.matmul(
                    accs[0:32, :],
                    selb[:, j * 32 : (j + 1) * 32],
                    expS[:, j * QS : (j + 1) * QS],
                    start=(j == 0),
                    stop=(j == npt - 1),
                )
            return expS, accs

        def unitB(h, t, expS, accs, attn_t):
            npt = NP_T[t]
            lo = LO[t]
            acco = psSp.tile([128, 512], F32, name="acco", tag="psS")
            rc = rcpp.tile([32, 512], F32R, tag="rcp")
            rs1 = rcpp.tile([32, 512], F32, tag="rcs1")
            rs2 = rcpp.tile([32, 512], F32, tag="rcs2")
            nc.vector.reciprocal_approx_accurate(rs2[:], accs[0:32, :], rs1[:])
            nc.vector.tensor_mul(rc[:], rs2[:], vm[:, t * QS : (t + 1) * QS])
            for j in range(npt):
                bt = flexp.tile([128, 512], F32, tag="flex")
                nc.tensor.matmul(
                    bt[:],
                    bds[0 : 2 * npt, j * 128 : (j + 1) * 128],
                    rc[0 : 2 * npt, :],
                    start=True,
                    stop=True,
                )
                pt = ptp.tile([128, 512], F32R, tag="pt")
                nc.vector.tensor_mul(pt[:], expS[:, j * QS : (j + 1) * QS], bt[:])
                cp = lo // 2 + j
                nc.tensor.matmul(
                    acco[0:64, :],
                    vv[:, cp * F + h * 64 : cp * F + h * 64 + 64],
                    pt[:],
                    start=(j == 0),
                    stop=(j == npt - 1),
                )
            nc.scalar.copy(attn_t[:, :], acco[0:64, :])

        def outproj(t, atts):
            for sc2 in range(4):
                ob = outp.tile([128, 1024], F32, tag="outsb")
                for eh in range(2):
                    po = flexp.tile([128, 512], F32, tag="flex")
                    for h in range(HPC):
                        nc.tensor.matmul(
                            po[:],
                            atts[h][:, sc2 * 128 : sc2 * 128 + 128],
                            wo[:, h * E + eh * 512 : h * E + eh * 512 + 512],
                            start=(h == 0),
                            stop=(h == HPC - 1),
                        )
                    nc.scalar.copy(ob[:, eh * 512 : (eh + 1) * 512], po[:])
                row = (4 * t + sc2) * 128
                nc.gpsimd.dma_start(out_d.ap()[row : row + 128, :], ob[:])

        def qproj(sc4):
            pqs = [
                psSp.tile([128, 512], F32, name=f"pq{fold}", tag="psS")
                for fold in range(2)
            ]
            for e in range(8):
                xt = xqp.tile([128, 512], F32R, tag="xq")
                nc.sync.dma_start(
                    xt[:],
                    xq_d.ap()[e * 128 : (e + 1) * 128, sc4 * 512 : (sc4 + 1) * 512],
                )
                for fold in range(2):
                    nc.tensor.matmul(
                        pqs[fold][:],
                        wq[:, e * F + fold * 128 : e * F + fold * 128 + 128],
                        xt[:],
                        start=(e == 0),
                        stop=(e == 7),
                    )
            for fold in range(2):
                nc.scalar.copy(
                    qT[:, fold * S + sc4 * 512 : fold * S + (sc4 + 1) * 512],
                    pqs[fold][:],
                )

        units = [(t, h) for t in range(T_SLABS) for h in range(HPC)]
        pending = None
        atts_by_t = {t: [] for t in range(T_SLABS)}
        for t, h in units:
            if h == 0:
                qproj(t)
            stA = unitA(h, t)
            if pending is not None:
                pt_, ph_, pexpS, pacc, pattn = pending
                unitB(ph_, pt_, pexpS, pacc, pattn)
                atts_by_t[pt_].append(pattn)
                if debug:
                    nc.gpsimd.dma_start(
                        at_d.ap()[ph_ * 64 : ph_ * 64 + 64, pt_ * QS : (pt_ + 1) * QS],
                        pattn[:],
                    )
                if len(atts_by_t[pt_]) == HPC:
                    outproj(pt_, atts_by_t[pt_])
            attn_t = attp.tile([64, 512], F32R, tag="att")
            pending = (t, h, stA[0], stA[1], attn_t)
        pt_, ph_, pexpS, pacc, pattn = pending
        unitB(ph_, pt_, pexpS, pacc, pattn)
        atts_by_t[pt_].append(pattn)
        if debug:
            nc.gpsimd.dma_start(
                at_d.ap()[ph_ * 64 : ph_ * 64 + 64, pt_ * QS : (pt_ + 1) * QS],
                pattn[:],
            )
        outproj(pt_, atts_by_t[pt_])

        if debug:
            nc.gpsimd.dma_start(qT_d.ap(), qT[:])
            nc.gpsimd.dma_start(kT_d.ap(), kT[:])
            nc.gpsimd.dma_start(vv_d.ap(), vv[:])

    nc.compile()
    return nc


_NC_CACHE = []


def _get_nc():
    if not _NC_CACHE:
        _NC_CACHE.append(build_nc())
    return _NC_CACHE[0]


def _host_consts():
    selc = np.zeros((128, MAXP * 32), np.float32)
    for k in range(128):
        for j in range(MAXP):
            selc[k, j * 32 + 2 * j + k // 64] = 1.0
    bdsel = np.zeros((32, MAXP * 128), np.float32)
    for j in range(MAXP):
        for p in range(128):
            bdsel[2 * j + p // 64, j * 128 + p] = 1.0
    vmask = np.zeros((32, T_SLABS * QS), np.float32)
    for t in range(T_SLABS):
        for m in range(2 * NP_T[t]):
            c = LO[t] + m
            for qb in range(QS // BLK):
                r = 8 * t + qb
                if abs(r - c) <= BAND:
                    vmask[m, t * QS + qb * 64 : t * QS + (qb + 1) * 64] = 1.0
    return selc, bdsel, vmask


def kernel(query, key, value, Wq, Wk, Wv, Wo):
    query = np.asarray(query, np.float32)
    key = np.asarray(key, np.float32)
    value = np.asarray(value, np.float32)
    Wq = np.asarray(Wq, np.float32)
    Wk = np.asarray(Wk, np.float32)
    Wv = np.asarray(Wv, np.float32)
    Wo = np.asarray(Wo, np.float32)

    nc = _get_nc()
    selc, bdsel, vmask = _host_consts()

    in_maps = []
    for c in range(NCORES):
        b, g = divmod(c, HPC)
        fs = slice(F * g, F * (g + 1))
        in_maps.append(
            {
                "xqT": np.ascontiguousarray(query[b].T),
                "xkT": np.ascontiguousarray(key[b].T),
                "xvT": np.ascontiguousarray(value[b].T),
                "wqT": np.ascontiguousarray((Wq[fs, :] * SCALE).T),
                "wkT": np.ascontiguousarray(Wk[fs, :].T),
                "wvT": np.ascontiguousarray(Wv[fs, :].T),
                "woT": np.ascontiguousarray(Wo[:, fs].T),
                "selc": selc,
                "bdsel": bdsel,
                "vmask": vmask,
            }
        )

    res = bass_utils.run_bass_kernel_spmd(nc, in_maps, core_ids=list(range(NCORES)))
    out = np.zeros((B, S, E), np.float32)
    for c in range(NCORES):
        b = c // HPC
        out[b] += res.results[c]["out"]
    return out



# revision 5
# speedup vs baseline: 1.0329x; 1.0329x over previous
"""Block-sparse (banded) attention kernel for Trainium2, 8 NeuronCores.

Sharding: data-parallel over batch (2) x tensor-parallel over heads
(16 heads -> 4 per core).  Each core computes its 4 heads' Q/K/V
projections, banded block attention (|r-c| <= 15 blocks, per-block
softmax), and a partial output projection; the host sums the 4 partial
outputs per batch element.

All matmul operands are bf16 (PSUM accumulation stays fp32).  Heads are
processed in pairs per fold so the K=64 score matmuls, M=32 row-sum
matmuls, K<=32 broadcast matmuls and M=64 attn@V matmuls each run as
two concurrent tile_position'd matmuls on the PE array.

Self-contained: hardcodes all shapes; only needs the concourse tree that
the environment already puts on sys.path.
"""

import sys

for _p in ("/opt/trn_rl_repo",):
    if _p not in sys.path:
        sys.path.insert(0, _p)

from contextlib import ExitStack

import numpy as np
import ml_dtypes

import concourse.bacc as bacc
import concourse.tile as tile
from concourse import bass_utils, mybir

F32 = mybir.dt.float32
BF16 = mybir.dt.bfloat16
EXP = mybir.ActivationFunctionType.Exp

B, S, E = 2, 2048, 1024
H, HD, BLK = 16, 64, 64
NB = S // BLK  # 32 blocks
NCORES = 8
HPC = 4  # heads per core
F = HPC * HD  # 256 local features
BAND = 15
SCALE = HD ** -0.5
BFD = ml_dtypes.bfloat16

# per r8-slab (8 query blocks, q=512) column-block ranges, even-extended
T_SLABS = 4
QS = 512  # q extent per slab
LO = []
NP_T = []
for _t in range(T_SLABS):
    lo = max(0, 8 * _t - BAND)
    hi = min(NB - 1, 8 * _t + 7 + BAND)
    if (hi - lo + 1) % 2 == 1:
        if lo > 0:
            lo -= 1
        else:
            hi += 1
    LO.append(lo)
    NP_T.append((hi - lo + 1) // 2)
MAXP = max(NP_T)  # 16 pairs


def build_nc():
    nc = bacc.Bacc("TRN2", target_bir_lowering=False, debug=False)

    xq_d = nc.dram_tensor("xqT", [E, S], BF16, kind="ExternalInput")
    xk_d = nc.dram_tensor("xkT", [E, S], BF16, kind="ExternalInput")
    xv_d = nc.dram_tensor("xvT", [E, S], BF16, kind="ExternalInput")
    wq_d = nc.dram_tensor("wqT", [E, F], BF16, kind="ExternalInput")
    wk_d = nc.dram_tensor("wkT", [E, F], BF16, kind="ExternalInput")
    wv_d = nc.dram_tensor("wvT", [E, F], BF16, kind="ExternalInput")
    wo_d = nc.dram_tensor("woT", [F, E], BF16, kind="ExternalInput")
    sel_d = nc.dram_tensor("selc", [128, MAXP * 32], BF16, kind="ExternalInput")
    bds_d = nc.dram_tensor("bdsel", [64, MAXP * 128], BF16, kind="ExternalInput")
    vm_d = nc.dram_tensor("vmask", [64, T_SLABS * QS], BF16, kind="ExternalInput")
    out_d = nc.dram_tensor("out", [S, E], F32, kind="ExternalOutput")

    with tile.TileContext(nc) as tc, ExitStack() as ctx, nc.allow_low_precision(
        reason="bf16 pipeline; fp32 PSUM accumulation throughout"
    ):
        pers = ctx.enter_context(tc.tile_pool(name="pers", bufs=1))
        qT = pers.tile([128, 2 * S], BF16, tag="qT")
        kT = pers.tile([128, 2 * S], BF16, tag="kT")
        vv = pers.tile([128, 16 * F], BF16, tag="vv")
        wq = pers.tile([128, 8 * F], BF16, tag="wq")
        wk = pers.tile([128, 8 * F], BF16, tag="wk")
        wv = pers.tile([128, 8 * F], BF16, tag="wv")
        wo = pers.tile([128, 2 * E], BF16, tag="wo")
        selb = pers.tile([128, MAXP * 32], BF16, tag="selb")
        bds = pers.tile([64, MAXP * 128], BF16, tag="bds")
        vm = pers.tile([64, T_SLABS * QS], BF16, tag="vm")

        # k-projection weights first: phase 1 is on the critical path
        nc.sync.dma_start(
            wk[:].rearrange("p (c f) -> p c f", c=8),
            wk_d.ap().rearrange("(c p) f -> p c f", p=128),
        )
        # remaining weights/constants arrive via gpsimd (SWDGE) so they don't
        # queue ahead of the phase-1/2 x-tile loads on the sync ring
        nc.gpsimd.dma_start(
            wv[:].rearrange("p (c f) -> p c f", c=8),
            wv_d.ap().rearrange("(c p) f -> p c f", p=128),
        )
        nc.gpsimd.dma_start(
            wq[:].rearrange("p (c f) -> p c f", c=8),
            wq_d.ap().rearrange("(c p) f -> p c f", p=128),
        )
        nc.gpsimd.dma_start(
            wo[:].rearrange("p (c e) -> p c e", c=2),
            wo_d.ap().rearrange("(c p) e -> p c e", p=128),
        )
        nc.gpsimd.dma_start(selb[:], sel_d.ap())
        nc.gpsimd.dma_start(bds[:], bds_d.ap())
        nc.gpsimd.dma_start(vm[:], vm_d.ap())

        # ---- phase 1: k projection (kT layout [f, s]) ----
        with tc.tile_pool(name="xk", bufs=2) as xkp, tc.tile_pool(
            name="psK", bufs=1, space="PSUM"
        ) as pskp:
            psK = pskp.tile([128, 4096], F32)
            for e in range(8):
                xt = xkp.tile([128, S], BF16, tag="xk")
                nc.sync.dma_start(xt[:], xk_d.ap()[e * 128 : (e + 1) * 128, :])
                for fold in range(2):
                    for sc in range(4):
                        nc.tensor.matmul(
                            psK[:, (fold * 4 + sc) * 512 : (fold * 4 + sc + 1) * 512],
                            wk[:, e * F + fold * 128 : e * F + fold * 128 + 128],
                            xt[:, sc * 512 : (sc + 1) * 512],
                            start=(e == 0),
                            stop=(e == 7),
                        )
            for fold in range(2):
                for sc in range(4):
                    nc.vector.tensor_copy(
                        kT[:, fold * S + sc * 512 : fold * S + (sc + 1) * 512],
                        psK[:, (fold * 4 + sc) * 512 : (fold * 4 + sc + 1) * 512],
                    )

        # ---- phase 2: v projection (natural layout [s, f]) ----
        with tc.tile_pool(name="xv", bufs=3) as xvp, tc.tile_pool(
            name="psV", bufs=2, space="PSUM"
        ) as psvp:
            for sc in range(4):
                # one PSUM bank per sub-chunk: accumulation groups must not
                # interleave within a bank
                pvs = [
                    psvp.tile([128, 256], F32, name=f"pv{sub}", tag=f"psV{sub}")
                    for sub in range(4)
                ]
                for e in range(8):
                    xt = xvp.tile([128, 512], BF16, tag="xv")
                    nc.sync.dma_start(
                        xt[:],
                        xv_d.ap()[e * 128 : (e + 1) * 128, sc * 512 : (sc + 1) * 512],
                    )
                    for sub in range(4):
                        nc.tensor.matmul(
                            pvs[sub][:],
                            xt[:, sub * 128 : (sub + 1) * 128],
                            wv[:, e * F : (e + 1) * F],
                            start=(e == 0),
                            stop=(e == 7),
                        )
                for sub in range(4):
                    nc.vector.tensor_copy(
                        vv[:, sc * 1024 + sub * 256 : sc * 1024 + (sub + 1) * 256],
                        pvs[sub][:],
                    )

        # ---- phase 3: q projection + attention + output projection ----
        xqp = ctx.enter_context(tc.tile_pool(name="xq", bufs=3))
        psSp = ctx.enter_context(tc.tile_pool(name="psS", bufs=3, space="PSUM"))
        accp = ctx.enter_context(tc.tile_pool(name="accP", bufs=3, space="PSUM"))
        flexp = ctx.enter_context(tc.tile_pool(name="flex", bufs=2, space="PSUM"))
        expp = ctx.enter_context(tc.tile_pool(name="expS", bufs=4))
        ptp = ctx.enter_context(tc.tile_pool(name="pt", bufs=4))
        rcpp = ctx.enter_context(tc.tile_pool(name="rcp", bufs=2))
        attp = ctx.enter_context(tc.tile_pool(name="att", bufs=4))
        outp = ctx.enter_context(tc.tile_pool(name="outsb", bufs=2))

        def unitA(f, t):
            """Scores + exp + per-block row sums for head pair (2f, 2f+1)."""
            npt = NP_T[t]
            lo = LO[t]
            expS0 = expp.tile([128, MAXP * QS], BF16, tag="expS")
            expS1 = expp.tile([128, MAXP * QS], BF16, tag="expS")
            accs = accp.tile([128, 512], F32, tag="accP")
            for j in range(npt):
                c0 = lo + 2 * j
                psA = psSp.tile([128, 512], F32, name="psA", tag="psS")
                psB = psSp.tile([128, 512], F32, name="psB", tag="psS")
                # row-tiled concurrent score matmuls: head 2f on PE rows
                # 0-63, head 2f+1 on rows 64-127
                nc.tensor.matmul(
                    psA[:],
                    kT[0:64, f * S + c0 * 64 : f * S + c0 * 64 + 128],
                    qT[0:64, f * S + t * QS : f * S + (t + 1) * QS],
                    start=True,
                    stop=True,
                )
                nc.tensor.matmul(
                    psB[:],
                    kT[64:128, f * S + c0 * 64 : f * S + c0 * 64 + 128],
                    qT[64:128, f * S + t * QS : f * S + (t + 1) * QS],
                    start=True,
                    stop=True,
                )
                nc.scalar.activation(expS0[:, j * QS : (j + 1) * QS], psA[:], EXP)
                nc.scalar.activation(expS1[:, j * QS : (j + 1) * QS], psB[:], EXP)
                # col-tiled concurrent row-sum matmuls: head0 denominators in
                # accs rows 0-31, head1 in rows 32-63 (disjoint partitions, so
                # the interleaved accumulation groups are race-free)
                nc.tensor.matmul(
                    accs[0:32, :],
                    selb[:, j * 32 : (j + 1) * 32],
                    expS0[:, j * QS : (j + 1) * QS],
                    start=(j == 0),
                    stop=(j == npt - 1),
                    skip_group_check=True,
                )
                nc.tensor.matmul(
                    accs[32:64, :],
                    selb[:, j * 32 : (j + 1) * 32],
                    expS1[:, j * QS : (j + 1) * QS],
                    start=(j == 0),
                    stop=(j == npt - 1),
                    skip_group_check=True,
                )
            return expS0, expS1, accs

        def unitB(f, t, expS0, expS1, accs, attn_t):
            npt = NP_T[t]
            lo = LO[t]
            rc = rcpp.tile([64, 512], BF16, tag="rcp")
            rs1 = rcpp.tile([64, 512], F32, tag="rcs1")
            rs2 = rcpp.tile([64, 512], F32, tag="rcs2")
            nc.vector.reciprocal_approx_accurate(rs2[:], accs[0:64, :], rs1[:])
            nc.vector.tensor_mul(rc[:], rs2[:], vm[:, t * QS : (t + 1) * QS])
            acco = accp.tile([128, 512], F32, tag="accP")
            for j in range(npt):
                bt0 = flexp.tile([128, 512], F32, name="bt0", tag="flex")
                bt1 = flexp.tile([128, 512], F32, name="bt1", tag="flex")
                # row-tiled concurrent broadcast matmuls (K=2*npt <= 32):
                # head0 reciprocal rows at partitions 0.., head1 at 32..
                nc.tensor.matmul(
                    bt0[:],
                    bds[0 : 2 * npt, j * 128 : (j + 1) * 128],
                    rc[0 : 2 * npt, :],
                    start=True,
                    stop=True,
                )
                nc.tensor.matmul(
                    bt1[:],
                    bds[32 : 32 + 2 * npt, j * 128 : (j + 1) * 128],
                    rc[32 : 32 + 2 * npt, :],
                    start=True,
                    stop=True,
                )
                pt0 = ptp.tile([128, 512], BF16, tag="pt")
                pt1 = ptp.tile([128, 512], BF16, tag="pt")
                nc.vector.tensor_mul(pt0[:], expS0[:, j * QS : (j + 1) * QS], bt0[:])
                nc.vector.tensor_mul(pt1[:], expS1[:, j * QS : (j + 1) * QS], bt1[:])
                cp = lo // 2 + j
                # col-tiled concurrent attn@V matmuls into one PSUM bank:
                # head0 output rows 0-63, head1 rows 64-127
                nc.tensor.matmul(
                    acco[0:64, :],
                    vv[:, cp * F + (2 * f) * 64 : cp * F + (2 * f) * 64 + 64],
                    pt0[:],
                    start=(j == 0),
                    stop=(j == npt - 1),
                    skip_group_check=True,
                )
                nc.tensor.matmul(
                    acco[64:128, :],
                    vv[:, cp * F + (2 * f + 1) * 64 : cp * F + (2 * f + 1) * 64 + 64],
                    pt1[:],
                    start=(j == 0),
                    stop=(j == npt - 1),
                    skip_group_check=True,
                )
            # [128 local-feature rows, 512 q] bf16 — the lhsT layout the
            # K=128 output projection wants
            nc.scalar.copy(attn_t[:, :], acco[:])

        def outproj(t, atts):
            for sc2 in range(4):
                ob = outp.tile([128, 1024], F32, tag="outsb")
                for eh in range(2):
                    po = psSp.tile([128, 512], F32, name="po", tag="psS")
                    for f in range(2):
                        nc.tensor.matmul(
                            po[:],
                            atts[f][:, sc2 * 128 : sc2 * 128 + 128],
                            wo[:, f * E + eh * 512 : f * E + eh * 512 + 512],
                            start=(f == 0),
                            stop=(f == 1),
                        )
                    nc.vector.tensor_copy(ob[:, eh * 512 : (eh + 1) * 512], po[:])
                row = (4 * t + sc2) * 128
                nc.gpsimd.dma_start(out_d.ap()[row : row + 128, :], ob[:])

        def qproj(sc4):
            pqs = [
                psSp.tile([128, 512], F32, name=f"pq{fold}", tag="psS")
                for fold in range(2)
            ]
            for e in range(8):
                xt = xqp.tile([128, 512], BF16, tag="xq")
                nc.sync.dma_start(
                    xt[:],
                    xq_d.ap()[e * 128 : (e + 1) * 128, sc4 * 512 : (sc4 + 1) * 512],
                )
                for fold in range(2):
                    nc.tensor.matmul(
                        pqs[fold][:],
                        wq[:, e * F + fold * 128 : e * F + fold * 128 + 128],
                        xt[:],
                        start=(e == 0),
                        stop=(e == 7),
                    )
            for fold in range(2):
                nc.vector.tensor_copy(
                    qT[:, fold * S + sc4 * 512 : fold * S + (sc4 + 1) * 512],
                    pqs[fold][:],
                )

        units = [(t, f) for t in range(T_SLABS) for f in range(2)]
        pending = None
        atts_by_t = {t: [] for t in range(T_SLABS)}
        for t, f in units:
            if f == 0:
                qproj(t)
            stA = unitA(f, t)
            if pending is not None:
                pt_, pf_, pe0, pe1, pacc, pattn = pending
                unitB(pf_, pt_, pe0, pe1, pacc, pattn)
                atts_by_t[pt_].append(pattn)
                if len(atts_by_t[pt_]) == 2:
                    outproj(pt_, atts_by_t[pt_])
            attn_t = attp.tile([128, 512], BF16, tag="att")
            pending = (t, f, stA[0], stA[1], stA[2], attn_t)
        pt_, pf_, pe0, pe1, pacc, pattn = pending
        unitB(pf_, pt_, pe0, pe1, pacc, pattn)
        atts_by_t[pt_].append(pattn)
        outproj(pt_, atts_by_t[pt_])

    nc.compile()
    return nc


_NC_CACHE = []


def _get_nc():
    if not _NC_CACHE:
        _NC_CACHE.append(build_nc())
    return _NC_CACHE[0]


def _host_consts():
    selc = np.zeros((128, MAXP * 32), np.float32)
    for k in range(128):
        for j in range(MAXP):
            selc[k, j * 32 + 2 * j + k // 64] = 1.0
    bdsel = np.zeros((64, MAXP * 128), np.float32)
    for j in range(MAXP):
        for p in range(128):
            bdsel[2 * j + p // 64, j * 128 + p] = 1.0
            bdsel[32 + 2 * j + p // 64, j * 128 + p] = 1.0
    vmask = np.zeros((64, T_SLABS * QS), np.float32)
    for t in range(T_SLABS):
        for m in range(2 * NP_T[t]):
            c = LO[t] + m
            for qb in range(QS // BLK):
                r = 8 * t + qb
                if abs(r - c) <= BAND:
                    vmask[m, t * QS + qb * 64 : t * QS + (qb + 1) * 64] = 1.0
                    vmask[32 + m, t * QS + qb * 64 : t * QS + (qb + 1) * 64] = 1.0
    return (
        selc.astype(BFD),
        bdsel.astype(BFD),
        vmask.astype(BFD),
    )


def build_in_maps(query, key, value, Wq, Wk, Wv, Wo):
    query = np.asarray(query, np.float32)
    key = np.asarray(key, np.float32)
    value = np.asarray(value, np.float32)
    Wq = np.asarray(Wq, np.float32)
    Wk = np.asarray(Wk, np.float32)
    Wv = np.asarray(Wv, np.float32)
    Wo = np.asarray(Wo, np.float32)

    selc, bdsel, vmask = _host_consts()
    xs = [np.ascontiguousarray(a[b].T).astype(BFD) for a in (query, key, value) for b in range(B)]
    in_maps = []
    for c in range(NCORES):
        b, g = divmod(c, HPC)
        fs = slice(F * g, F * (g + 1))
        in_maps.append(
            {
                "xqT": xs[0 + b],
                "xkT": xs[2 + b],
                "xvT": xs[4 + b],
                "wqT": np.ascontiguousarray((Wq[fs, :] * SCALE).T).astype(BFD),
                "wkT": np.ascontiguousarray(Wk[fs, :].T).astype(BFD),
                "wvT": np.ascontiguousarray(Wv[fs, :].T).astype(BFD),
                "woT": np.ascontiguousarray(Wo[:, fs].T).astype(BFD),
                "selc": selc,
                "bdsel": bdsel,
                "vmask": vmask,
            }
        )
    return in_maps


def kernel(query, key, value, Wq, Wk, Wv, Wo):
    nc = _get_nc()
    in_maps = build_in_maps(query, key, value, Wq, Wk, Wv, Wo)
    res = bass_utils.run_bass_kernel_spmd(nc, in_maps, core_ids=list(range(NCORES)))
    out = np.zeros((B, S, E), np.float32)
    for c in range(NCORES):
        b = c // HPC
        out[b] += res.results[c]["out"]
    return out


# revision 11
# speedup vs baseline: 1.3225x; 1.2803x over previous
"""Block-sparse (banded) attention kernel for Trainium2, 8 NeuronCores.

Sharding: data-parallel over batch (2) x tensor-parallel over heads
(16 heads -> 4 per core).  Each core computes its 4 heads' Q/K/V
projections, banded block attention (|r-c| <= 15 blocks, per-block
softmax), and a partial output projection; the host sums the 4 partial
outputs per batch element.

All matmul operands are bf16 (PSUM accumulation stays fp32).  Heads are
processed in pairs per fold; scores / row-sum / broadcast / attn@V
matmuls run as concurrent tile_position'd pairs.  Phase 3 merges the
scores pipeline of unit u with the value pipeline of unit u-1 and the
output projection of the previous slab at macro-step granularity so the
in-order PE queue never chains behind ACT/DVE latency.

Self-contained: hardcodes all shapes; only needs the concourse tree that
the environment already puts on sys.path.
"""

import sys

for _p in ("/opt/trn_rl_repo",):
    if _p not in sys.path:
        sys.path.insert(0, _p)

from contextlib import ExitStack

import numpy as np
import ml_dtypes

import concourse.bacc as bacc
import concourse.tile as tile
from concourse import bass_utils, mybir

F32 = mybir.dt.float32
BF16 = mybir.dt.bfloat16
EXP = mybir.ActivationFunctionType.Exp

B, S, E = 2, 2048, 1024
H, HD, BLK = 16, 64, 64
NB = S // BLK  # 32 blocks
NCORES = 8
HPC = 4  # heads per core
F = HPC * HD  # 256 local features
BAND = 15
SCALE = HD ** -0.5
BFD = ml_dtypes.bfloat16

# per r8-slab (8 query blocks, q=512) column-block ranges, even-extended
T_SLABS = 4
QS = 512  # q extent per slab
LO = []
NP_T = []
for _t in range(T_SLABS):
    lo = max(0, 8 * _t - BAND)
    hi = min(NB - 1, 8 * _t + 7 + BAND)
    if (hi - lo + 1) % 2 == 1:
        if lo > 0:
            lo -= 1
        else:
            hi += 1
    LO.append(lo)
    NP_T.append((hi - lo + 1) // 2)
MAXP = max(NP_T)  # 16 pairs


def build_nc():
    nc = bacc.Bacc("TRN2", target_bir_lowering=False, debug=False)

    xq_d = nc.dram_tensor("xqT", [E, S], BF16, kind="ExternalInput")
    xk_d = nc.dram_tensor("xkT", [E, S], BF16, kind="ExternalInput")
    xv_d = nc.dram_tensor("xvT", [E, S], BF16, kind="ExternalInput")
    wq_d = nc.dram_tensor("wqT", [E, F], BF16, kind="ExternalInput")
    wk_d = nc.dram_tensor("wkT", [E, F], BF16, kind="ExternalInput")
    wv_d = nc.dram_tensor("wvT", [E, F], BF16, kind="ExternalInput")
    wo_d = nc.dram_tensor("woT", [F, E], BF16, kind="ExternalInput")
    sel_d = nc.dram_tensor("selc", [128, MAXP * 32], BF16, kind="ExternalInput")
    bds_d = nc.dram_tensor("bdsel", [64, MAXP * 128], BF16, kind="ExternalInput")
    vm_d = nc.dram_tensor("vmask", [64, T_SLABS * QS], BF16, kind="ExternalInput")
    out_d = nc.dram_tensor("out", [S, E], F32, kind="ExternalOutput")

    with tile.TileContext(nc) as tc, ExitStack() as ctx, nc.allow_low_precision(
        reason="bf16 pipeline; fp32 PSUM accumulation throughout"
    ):
        pers = ctx.enter_context(tc.tile_pool(name="pers", bufs=1))
        qT = pers.tile([128, 2 * S], BF16, tag="qT")
        kT = pers.tile([128, 2 * S], BF16, tag="kT")
        vv = pers.tile([128, 16 * F], BF16, tag="vv")
        wq = pers.tile([128, 8 * F], BF16, tag="wq")
        wk = pers.tile([128, 8 * F], BF16, tag="wk")
        wv = pers.tile([128, 8 * F], BF16, tag="wv")
        wo = pers.tile([128, 2 * E], BF16, tag="wo")
        selb = pers.tile([128, MAXP * 32], BF16, tag="selb")
        bds = pers.tile([64, MAXP * 128], BF16, tag="bds")
        vm = pers.tile([64, T_SLABS * QS], BF16, tag="vm")

        # k-projection weights first: phase 1 is on the critical path
        nc.sync.dma_start(
            wk[:].rearrange("p (c f) -> p c f", c=8),
            wk_d.ap().rearrange("(c p) f -> p c f", p=128),
        )
        # remaining weights/constants arrive via gpsimd (SWDGE) so they don't
        # queue ahead of the phase-1/2 x-tile loads on the sync ring
        nc.gpsimd.dma_start(
            wv[:].rearrange("p (c f) -> p c f", c=8),
            wv_d.ap().rearrange("(c p) f -> p c f", p=128),
        )
        nc.gpsimd.dma_start(
            wq[:].rearrange("p (c f) -> p c f", c=8),
            wq_d.ap().rearrange("(c p) f -> p c f", p=128),
        )
        nc.gpsimd.dma_start(
            wo[:].rearrange("p (c e) -> p c e", c=2),
            wo_d.ap().rearrange("(c p) e -> p c e", p=128),
        )
        nc.gpsimd.dma_start(selb[:], sel_d.ap())
        nc.gpsimd.dma_start(bds[:], bds_d.ap())
        nc.gpsimd.dma_start(vm[:], vm_d.ap())

        # ---- phase 1: k projection (kT layout [f, s]) ----
        with tc.tile_pool(name="xk", bufs=2) as xkp, tc.tile_pool(
            name="psK", bufs=1, space="PSUM"
        ) as pskp:
            psK = pskp.tile([128, 4096], F32)
            for e in range(8):
                xt = xkp.tile([128, S], BF16, tag="xk")
                nc.sync.dma_start(xt[:], xk_d.ap()[e * 128 : (e + 1) * 128, :])
                for fold in range(2):
                    for sc in range(4):
                        nc.tensor.matmul(
                            psK[:, (fold * 4 + sc) * 512 : (fold * 4 + sc + 1) * 512],
                            wk[:, e * F + fold * 128 : e * F + fold * 128 + 128],
                            xt[:, sc * 512 : (sc + 1) * 512],
                            start=(e == 0),
                            stop=(e == 7),
                        )
            for fold in range(2):
                for sc in range(4):
                    nc.scalar.copy(
                        kT[:, fold * S + sc * 512 : fold * S + (sc + 1) * 512],
                        psK[:, (fold * 4 + sc) * 512 : (fold * 4 + sc + 1) * 512],
                    )

        # ---- phase 2: v projection (natural layout [s, f]) ----
        with tc.tile_pool(name="xv", bufs=3) as xvp, tc.tile_pool(
            name="psV", bufs=2, space="PSUM"
        ) as psvp:
            for sc in range(4):
                # one PSUM bank per sub-chunk: accumulation groups must not
                # interleave within a bank
                pvs = [
                    psvp.tile([128, 256], F32, name=f"pv{sub}", tag=f"psV{sub}")
                    for sub in range(4)
                ]
                for e in range(8):
                    xt = xvp.tile([128, 512], BF16, tag="xv")
                    nc.sync.dma_start(
                        xt[:],
                        xv_d.ap()[e * 128 : (e + 1) * 128, sc * 512 : (sc + 1) * 512],
                    )
                    for sub in range(4):
                        nc.tensor.matmul(
                            pvs[sub][:],
                            xt[:, sub * 128 : (sub + 1) * 128],
                            wv[:, e * F : (e + 1) * F],
                            start=(e == 0),
                            stop=(e == 7),
                        )
                for sub in range(4):
                    nc.scalar.copy(
                        vv[:, sc * 1024 + sub * 256 : sc * 1024 + (sub + 1) * 256],
                        pvs[sub][:],
                    )

        # ---- phase 2.5: q projection for all four slabs ----
        with tc.tile_pool(name="xq", bufs=3) as xqp, tc.tile_pool(
            name="psQ", bufs=4, space="PSUM"
        ) as psqp:
            for sc4 in range(4):
                pqs = [
                    psqp.tile([128, 512], F32, name=f"pq{fold}", tag="psQ")
                    for fold in range(2)
                ]
                for e in range(8):
                    xt = xqp.tile([128, 512], BF16, tag="xq")
                    nc.sync.dma_start(
                        xt[:],
                        xq_d.ap()[e * 128 : (e + 1) * 128, sc4 * 512 : (sc4 + 1) * 512],
                    )
                    for fold in range(2):
                        nc.tensor.matmul(
                            pqs[fold][:],
                            wq[:, e * F + fold * 128 : e * F + fold * 128 + 128],
                            xt[:],
                            start=(e == 0),
                            stop=(e == 7),
                        )
                for fold in range(2):
                    nc.scalar.copy(
                        qT[:, fold * S + sc4 * 512 : fold * S + (sc4 + 1) * 512],
                        pqs[fold][:],
                    )

        # ---- phase 3: merged attention pipeline + output projection ----
        # PSUM budget (8 banks): psH [128,1024]x2 = 4, bt/po [128,1024]x1 = 2,
        # accs x1 = 1, acco x1 = 1.
        psHp = ctx.enter_context(tc.tile_pool(name="psH", bufs=2, space="PSUM"))
        btp = ctx.enter_context(tc.tile_pool(name="btP", bufs=1, space="PSUM"))
        accp = ctx.enter_context(tc.tile_pool(name="accP", bufs=1, space="PSUM"))
        accop = ctx.enter_context(tc.tile_pool(name="accoP", bufs=1, space="PSUM"))
        expp = ctx.enter_context(tc.tile_pool(name="expS", bufs=4))
        ptp = ctx.enter_context(tc.tile_pool(name="pt", bufs=8))
        rcpp = ctx.enter_context(tc.tile_pool(name="rcp", bufs=2))
        attp = ctx.enter_context(tc.tile_pool(name="att", bufs=4))
        outp = ctx.enter_context(tc.tile_pool(name="outsb", bufs=2))

        def stream_A(t, f):
            """Scores + exp + row sums for head pair (2f, 2f+1) of slab t.

            Returns (steps, shared) where shared collects the tiles stream_B
            needs.  Macro step k covers j pair (2k, 2k+1); row sums lag one
            macro so they never chain the PE behind the ACT exp latency.
            """
            npt = NP_T[t]
            lo = LO[t]
            mA = npt // 2
            shared = {}

            def alloc(_k=0):
                shared["expS0"] = expp.tile([128, MAXP * QS], BF16, name="expS0", tag="expS")
                shared["expS1"] = expp.tile([128, MAXP * QS], BF16, name="expS1", tag="expS")

            def scores(k):
                if k == 0:
                    alloc()
                for h, base in ((0, 0), (1, 64)):
                    ps = psHp.tile([128, 1024], F32, name="psH", tag="psH")
                    for jj in range(2):
                        j = 2 * k + jj
                        c0 = lo + 2 * j
                        nc.tensor.matmul(
                            ps[:, jj * 512 : (jj + 1) * 512],
                            kT[base : base + 64, f * S + c0 * 64 : f * S + c0 * 64 + 128],
                            qT[base : base + 64, f * S + t * QS : f * S + (t + 1) * QS],
                            start=True,
                            stop=True,
                        )
                    nc.scalar.activation(
                        shared["expS0" if h == 0 else "expS1"][
                            :, 2 * k * QS : (2 * k + 2) * QS
                        ],
                        ps[:],
                        EXP,
                    )

            def rowsums(k):
                if k == 0:
                    shared["accs"] = accp.tile([128, 512], F32, name="accs", tag="accP")
                accs = shared["accs"]
                for jj in range(2):
                    j = 2 * k + jj
                    nc.tensor.matmul(
                        accs[0:32, :],
                        selb[:, j * 32 : (j + 1) * 32],
                        shared["expS0"][:, j * QS : (j + 1) * QS],
                        start=(j == 0),
                        stop=(j == npt - 1),
                        skip_group_check=True,
                    )
                    nc.tensor.matmul(
                        accs[32:64, :],
                        selb[:, j * 32 : (j + 1) * 32],
                        shared["expS1"][:, j * QS : (j + 1) * QS],
                        start=(j == 0),
                        stop=(j == npt - 1),
                        skip_group_check=True,
                    )

            steps = []
            for k in range(mA):
                steps.append(lambda k=k: (scores(k), k > 0 and rowsums(k - 1)))
            steps.append(lambda: rowsums(mA - 1))
            return steps, shared

        def stream_B(t, f, shared, attn_t):
            """Reciprocal + broadcast + attn@V for the unit A just finished.

            V matmuls lag the bt/pt-mul macro by one so the PE never waits on
            the DVE/GPSIMD probability multiplies.  pt-muls split h0->DVE,
            h1->GPSIMD.
            """
            npt = NP_T[t]
            lo = LO[t]
            mB = npt // 2
            st = {}

            def recip():
                rc = rcpp.tile([64, 512], BF16, name="rc", tag="rcp")
                rs1 = rcpp.tile([64, 512], F32, name="rs1", tag="rcs1")
                rs2 = rcpp.tile([64, 512], F32, name="rs2", tag="rcs2")
                nc.vector.reciprocal_approx_accurate(rs2[:], shared["accs"][0:64, :], rs1[:])
                nc.vector.tensor_mul(rc[:], rs2[:], vm[:, t * QS : (t + 1) * QS])
                st["rc"] = rc

            def btmul(p):
                rc = st["rc"]
                pts = []
                for h in range(2):
                    hb = 32 * h
                    expS = shared["expS0" if h == 0 else "expS1"]
                    bt = btp.tile([128, 1024], F32, name="bt", tag="btP")
                    for jj in range(2):
                        j = 2 * p + jj
                        nc.tensor.matmul(
                            bt[:, jj * 512 : (jj + 1) * 512],
                            bds[hb : hb + 2 * npt, j * 128 : (j + 1) * 128],
                            rc[hb : hb + 2 * npt, :],
                            start=True,
                            stop=True,
                        )
                    ptt = ptp.tile([128, 1024], BF16, name="ptt", tag="pt")
                    nc.vector.tensor_mul(
                        ptt[:], expS[:, 2 * p * QS : (2 * p + 2) * QS], bt[:]
                    )
                    pts.append(ptt)
                st[("pt", p)] = pts

            def vmm(p):
                if p == 0:
                    st["acco"] = accop.tile([128, 512], F32, name="acco", tag="accoP")
                acco = st["acco"]
                pts = st.pop(("pt", p))
                for h in range(2):
                    for jj in range(2):
                        j = 2 * p + jj
                        cp = lo // 2 + j
                        nc.tensor.matmul(
                            acco[64 * h : 64 * h + 64, :],
                            vv[:, cp * F + (2 * f + h) * 64 : cp * F + (2 * f + h) * 64 + 64],
                            pts[h][:, jj * 512 : (jj + 1) * 512],
                            start=(j == 0),
                            stop=(j == npt - 1),
                            skip_group_check=True,
                        )

            steps = [recip]
            for p in range(mB):
                steps.append(lambda p=p: (btmul(p), p > 0 and vmm(p - 1)))
            steps.append(lambda: (vmm(mB - 1), nc.scalar.copy(attn_t[:], st["acco"][:])))
            return steps

        def stream_C(t, atts):
            """Output projection of slab t, dripped 2 matmuls per macro step."""
            st = {}

            def piece(sc2, eh):
                if eh == 0:
                    st["ob"] = outp.tile([128, 1024], F32, name="ob", tag="outsb")
                po = btp.tile([128, 1024], F32, name="po", tag="btP")
                for f in range(2):
                    nc.tensor.matmul(
                        po[:, 0:512],
                        atts[f][:, sc2 * 128 : sc2 * 128 + 128],
                        wo[:, f * E + eh * 512 : f * E + eh * 512 + 512],
                        start=(f == 0),
                        stop=(f == 1),
                    )
                nc.vector.tensor_copy(st["ob"][:, eh * 512 : (eh + 1) * 512], po[:, 0:512])
                if eh == 1:
                    row = (4 * t + sc2) * 128
                    nc.sync.dma_start(out_d.ap()[row : row + 128, :], st["ob"][:])

            return [lambda sc2=sc2, eh=eh: piece(sc2, eh) for sc2 in range(4) for eh in range(2)]

        def merge(streams):
            for k in range(max(len(s) for s in streams)):
                for s in streams:
                    if k < len(s):
                        s[k]()

        units = [(t, f) for t in range(T_SLABS) for f in range(2)]
        atts_by_t = {t: [] for t in range(T_SLABS)}
        pending_B = None
        for i, (t, f) in enumerate(units):
            sA, shared = stream_A(t, f)
            streams = [sA]
            if pending_B is not None:
                streams.append(pending_B)
            if f == 1 and t >= 1:
                streams.append(stream_C(t - 1, atts_by_t[t - 1]))
            merge(streams)
            attn_t = attp.tile([128, 512], BF16, name="attn_t", tag="att")
            atts_by_t[t].append(attn_t)
            pending_B = stream_B(t, f, shared, attn_t)
        merge([pending_B])
        merge([stream_C(T_SLABS - 1, atts_by_t[T_SLABS - 1])])

    nc.compile()
    return nc


_NC_CACHE = []


def _get_nc():
    if not _NC_CACHE:
        _NC_CACHE.append(build_nc())
    return _NC_CACHE[0]


def _host_consts():
    selc = np.zeros((128, MAXP * 32), np.float32)
    for k in range(128):
        for j in range(MAXP):
            selc[k, j * 32 + 2 * j + k // 64] = 1.0
    bdsel = np.zeros((64, MAXP * 128), np.float32)
    for j in range(MAXP):
        for p in range(128):
            bdsel[2 * j + p // 64, j * 128 + p] = 1.0
            bdsel[32 + 2 * j + p // 64, j * 128 + p] = 1.0
    vmask = np.zeros((64, T_SLABS * QS), np.float32)
    for t in range(T_SLABS):
        for m in range(2 * NP_T[t]):
            c = LO[t] + m
            for qb in range(QS // BLK):
                r = 8 * t + qb
                if abs(r - c) <= BAND:
                    vmask[m, t * QS + qb * 64 : t * QS + (qb + 1) * 64] = 1.0
                    vmask[32 + m, t * QS + qb * 64 : t * QS + (qb + 1) * 64] = 1.0
    return (
        selc.astype(BFD),
        bdsel.astype(BFD),
        vmask.astype(BFD),
    )


def build_in_maps(query, key, value, Wq, Wk, Wv, Wo):
    query = np.asarray(query, np.float32)
    key = np.asarray(key, np.float32)
    value = np.asarray(value, np.float32)
    Wq = np.asarray(Wq, np.float32)
    Wk = np.asarray(Wk, np.float32)
    Wv = np.asarray(Wv, np.float32)
    Wo = np.asarray(Wo, np.float32)

    selc, bdsel, vmask = _host_consts()
    xs = [np.ascontiguousarray(a[b].T).astype(BFD) for a in (query, key, value) for b in range(B)]
    in_maps = []
    for c in range(NCORES):
        b, g = divmod(c, HPC)
        fs = slice(F * g, F * (g + 1))
        in_maps.append(
            {
                "xqT": xs[0 + b],
                "xkT": xs[2 + b],
                "xvT": xs[4 + b],
                "wqT": np.ascontiguousarray((Wq[fs, :] * SCALE).T).astype(BFD),
                "wkT": np.ascontiguousarray(Wk[fs, :].T).astype(BFD),
                "wvT": np.ascontiguousarray(Wv[fs, :].T).astype(BFD),
                "woT": np.ascontiguousarray(Wo[:, fs].T).astype(BFD),
                "selc": selc,
                "bdsel": bdsel,
                "vmask": vmask,
            }
        )
    return in_maps


def kernel(query, key, value, Wq, Wk, Wv, Wo):
    nc = _get_nc()
    in_maps = build_in_maps(query, key, value, Wq, Wk, Wv, Wo)
    res = bass_utils.run_bass_kernel_spmd(nc, in_maps, core_ids=list(range(NCORES)))
    out = np.zeros((B, S, E), np.float32)
    for c in range(NCORES):
        b = c // HPC
        out[b] += res.results[c]["out"]
    return out


# revision 15
# speedup vs baseline: 1.6173x; 1.2229x over previous
"""Block-sparse (banded) attention kernel for Trainium2, 8 NeuronCores.

Sharding: data-parallel over batch (2) x tensor-parallel over heads
(16 heads -> 4 per core).  Each core computes its 4 heads' Q/K/V
projections, banded block attention (|r-c| <= 15 blocks, per-block
softmax), and a partial output projection; the host sums the 4 partial
outputs per batch element.

All matmul operands are bf16 (PSUM accumulation stays fp32).  Heads are
processed in pairs per fold; scores / row-sum / broadcast / attn@V
matmuls run as concurrent tile_position'd pairs.  Phase 3 merges the
scores pipeline of unit u with the value pipeline of unit u-1 and the
output projection of the previous slab at macro-step granularity so the
in-order PE queue never chains behind ACT/DVE latency.

Self-contained: hardcodes all shapes; only needs the concourse tree that
the environment already puts on sys.path.
"""

import sys

for _p in ("/opt/trn_rl_repo",):
    if _p not in sys.path:
        sys.path.insert(0, _p)

from contextlib import ExitStack

import numpy as np
import ml_dtypes

import concourse.bacc as bacc
import concourse.tile as tile
from concourse import bass_utils, mybir

F32 = mybir.dt.float32
BF16 = mybir.dt.bfloat16
EXP = mybir.ActivationFunctionType.Exp

B, S, E = 2, 2048, 1024
H, HD, BLK = 16, 64, 64
NB = S // BLK  # 32 blocks
NCORES = 8
HPC = 4  # heads per core
F = HPC * HD  # 256 local features
BAND = 15
SCALE = HD ** -0.5
BFD = ml_dtypes.bfloat16

# per r8-slab (8 query blocks, q=512) column-block ranges, even-extended
T_SLABS = 4
QS = 512  # q extent per slab
LO = []
NP_T = []
for _t in range(T_SLABS):
    lo = max(0, 8 * _t - BAND)
    hi = min(NB - 1, 8 * _t + 7 + BAND)
    if (hi - lo + 1) % 2 == 1:
        if lo > 0:
            lo -= 1
        else:
            hi += 1
    LO.append(lo)
    NP_T.append((hi - lo + 1) // 2)
MAXP = max(NP_T)  # 16 pairs


def build_nc():
    nc = bacc.Bacc("TRN2", target_bir_lowering=False, debug=False)

    xq_d = nc.dram_tensor("xqT", [E, S], BF16, kind="ExternalInput")
    xk_d = nc.dram_tensor("xkT", [E, S], BF16, kind="ExternalInput")
    xv_d = nc.dram_tensor("xvT", [E, S], BF16, kind="ExternalInput")
    wq_d = nc.dram_tensor("wqT", [E, F], BF16, kind="ExternalInput")
    wk_d = nc.dram_tensor("wkT", [E, F], BF16, kind="ExternalInput")
    wv_d = nc.dram_tensor("wvT", [E, F], BF16, kind="ExternalInput")
    wo_d = nc.dram_tensor("woT", [F, E], BF16, kind="ExternalInput")
    sel_d = nc.dram_tensor("selc", [128, MAXP * 32], BF16, kind="ExternalInput")
    bds_d = nc.dram_tensor("bdsel", [64, MAXP * 128], BF16, kind="ExternalInput")
    vm_d = nc.dram_tensor("vmask", [64, T_SLABS * QS], BF16, kind="ExternalInput")
    out_d = nc.dram_tensor("out", [S, E], F32, kind="ExternalOutput")

    with tile.TileContext(nc) as tc, ExitStack() as ctx, nc.allow_low_precision(
        reason="bf16 pipeline; fp32 PSUM accumulation throughout"
    ):
        pers = ctx.enter_context(tc.tile_pool(name="pers", bufs=1))
        qT = pers.tile([128, 2 * S], BF16, tag="qT")
        kT = pers.tile([128, 2 * S], BF16, tag="kT")
        vv = pers.tile([128, 16 * F], BF16, tag="vv")
        wq = pers.tile([128, 8 * F], BF16, tag="wq")
        wk = pers.tile([128, 8 * F], BF16, tag="wk")
        wv = pers.tile([128, 8 * F], BF16, tag="wv")
        wo = pers.tile([128, 2 * E], BF16, tag="wo")
        selb = pers.tile([128, MAXP * 32], BF16, tag="selb")
        bds = pers.tile([64, MAXP * 128], BF16, tag="bds")
        vm = pers.tile([64, T_SLABS * QS], BF16, tag="vm")

        # k-projection weights first: phase 1 is on the critical path
        nc.sync.dma_start(
            wk[:].rearrange("p (c f) -> p c f", c=8),
            wk_d.ap().rearrange("(c p) f -> p c f", p=128),
        )
        # remaining weights/constants arrive via gpsimd (SWDGE) so they don't
        # queue ahead of the phase-1/2 x-tile loads on the sync ring
        nc.gpsimd.dma_start(
            wv[:].rearrange("p (c f) -> p c f", c=8),
            wv_d.ap().rearrange("(c p) f -> p c f", p=128),
        )
        nc.gpsimd.dma_start(
            wq[:].rearrange("p (c f) -> p c f", c=8),
            wq_d.ap().rearrange("(c p) f -> p c f", p=128),
        )
        nc.gpsimd.dma_start(
            wo[:].rearrange("p (c e) -> p c e", c=2),
            wo_d.ap().rearrange("(c p) e -> p c e", p=128),
        )
        nc.gpsimd.dma_start(selb[:], sel_d.ap())
        nc.gpsimd.dma_start(bds[:], bds_d.ap())
        nc.gpsimd.dma_start(vm[:], vm_d.ap())

        # ---- phase 1: k projection (kT layout [f, s]) ----
        with tc.tile_pool(name="xk", bufs=2) as xkp, tc.tile_pool(
            name="psK", bufs=1, space="PSUM"
        ) as pskp:
            psK = pskp.tile([128, 4096], F32)
            for e in range(8):
                xt = xkp.tile([128, S], BF16, tag="xk")
                nc.sync.dma_start(xt[:], xk_d.ap()[e * 128 : (e + 1) * 128, :])
                for fold in range(2):
                    for sc in range(4):
                        nc.tensor.matmul(
                            psK[:, (fold * 4 + sc) * 512 : (fold * 4 + sc + 1) * 512],
                            wk[:, e * F + fold * 128 : e * F + fold * 128 + 128],
                            xt[:, sc * 512 : (sc + 1) * 512],
                            start=(e == 0),
                            stop=(e == 7),
                        )
            for fold in range(2):
                for sc in range(4):
                    nc.scalar.copy(
                        kT[:, fold * S + sc * 512 : fold * S + (sc + 1) * 512],
                        psK[:, (fold * 4 + sc) * 512 : (fold * 4 + sc + 1) * 512],
                    )

        # ---- phase 2: v projection (natural layout [s, f]) ----
        with tc.tile_pool(name="xv", bufs=3) as xvp, tc.tile_pool(
            name="psV", bufs=2, space="PSUM"
        ) as psvp:
            for sc in range(4):
                # one PSUM bank per sub-chunk: accumulation groups must not
                # interleave within a bank
                pvs = [
                    psvp.tile([128, 256], F32, name=f"pv{sub}", tag=f"psV{sub}")
                    for sub in range(4)
                ]
                for e in range(8):
                    xt = xvp.tile([128, 512], BF16, tag="xv")
                    nc.sync.dma_start(
                        xt[:],
                        xv_d.ap()[e * 128 : (e + 1) * 128, sc * 512 : (sc + 1) * 512],
                    )
                    for sub in range(4):
                        nc.tensor.matmul(
                            pvs[sub][:],
                            xt[:, sub * 128 : (sub + 1) * 128],
                            wv[:, e * F : (e + 1) * F],
                            start=(e == 0),
                            stop=(e == 7),
                        )
                for sub in range(4):
                    nc.scalar.copy(
                        vv[:, sc * 1024 + sub * 256 : sc * 1024 + (sub + 1) * 256],
                        pvs[sub][:],
                    )

        # ---- phase 2.5: q projection for all four slabs ----
        with tc.tile_pool(name="xq", bufs=3) as xqp, tc.tile_pool(
            name="psQ", bufs=4, space="PSUM"
        ) as psqp:
            for sc4 in range(4):
                pqs = [
                    psqp.tile([128, 512], F32, name=f"pq{fold}", tag="psQ")
                    for fold in range(2)
                ]
                for e in range(8):
                    xt = xqp.tile([128, 512], BF16, tag="xq")
                    nc.sync.dma_start(
                        xt[:],
                        xq_d.ap()[e * 128 : (e + 1) * 128, sc4 * 512 : (sc4 + 1) * 512],
                    )
                    for fold in range(2):
                        nc.tensor.matmul(
                            pqs[fold][:],
                            wq[:, e * F + fold * 128 : e * F + fold * 128 + 128],
                            xt[:],
                            start=(e == 0),
                            stop=(e == 7),
                        )
                for fold in range(2):
                    nc.scalar.copy(
                        qT[:, fold * S + sc4 * 512 : fold * S + (sc4 + 1) * 512],
                        pqs[fold][:],
                    )

        # ---- phase 3: merged attention pipeline + output projection ----
        # PSUM budget (8 banks): psH [128,512]x4 = 4, bt/po [128,512]x2 = 2,
        # accs x1 = 1, acco x1 = 1.  All cross-engine consumers lag their
        # producers by two j-steps so the PE issue stream never carries an
        # unsatisfied wait (keeps HAM at K=8/8).
        psHp = ctx.enter_context(tc.tile_pool(name="psH", bufs=4, space="PSUM"))
        btp = ctx.enter_context(tc.tile_pool(name="btP", bufs=2, space="PSUM"))
        accp = ctx.enter_context(tc.tile_pool(name="accP", bufs=1, space="PSUM"))
        accop = ctx.enter_context(tc.tile_pool(name="accoP", bufs=1, space="PSUM"))
        expp = ctx.enter_context(tc.tile_pool(name="expS", bufs=4))
        ptp = ctx.enter_context(tc.tile_pool(name="pt", bufs=8))
        rcpp = ctx.enter_context(tc.tile_pool(name="rcp", bufs=2))
        attp = ctx.enter_context(tc.tile_pool(name="att", bufs=4))
        outp = ctx.enter_context(tc.tile_pool(name="outsb", bufs=2))

        def stream_A(t, f):
            """Scores + exp + row sums for head pair (2f, 2f+1) of slab t.

            Returns (steps, shared) where shared collects the tiles stream_B
            needs.  Macro step k covers j pair (2k, 2k+1); row sums lag one
            macro so they never chain the PE behind the ACT exp latency.
            """
            npt = NP_T[t]
            lo = LO[t]
            mA = npt // 2
            shared = {}

            def alloc(_k=0):
                shared["expS0"] = expp.tile([128, MAXP * QS], BF16, name="expS0", tag="expS")
                shared["expS1"] = expp.tile([128, MAXP * QS], BF16, name="expS1", tag="expS")

            def scores(j):
                if j == 0:
                    alloc()
                c0 = lo + 2 * j
                for h, base in ((0, 0), (1, 64)):
                    ps = psHp.tile([128, 512], F32, name="psH", tag="psH")
                    nc.tensor.matmul(
                        ps[:],
                        kT[base : base + 64, f * S + c0 * 64 : f * S + c0 * 64 + 128],
                        qT[base : base + 64, f * S + t * QS : f * S + (t + 1) * QS],
                        start=True,
                        stop=True,
                    )
                    nc.scalar.activation(
                        shared["expS0" if h == 0 else "expS1"][:, j * QS : (j + 1) * QS],
                        ps[:],
                        EXP,
                    )

            def rowsums(j):
                if j == 0:
                    shared["accs"] = accp.tile([128, 512], F32, name="accs", tag="accP")
                accs = shared["accs"]
                for h, expS in ((0, shared["expS0"]), (1, shared["expS1"])):
                    nc.tensor.matmul(
                        accs[32 * h : 32 * h + 32, :],
                        selb[:, j * 32 : (j + 1) * 32],
                        expS[:, j * QS : (j + 1) * QS],
                        start=(j == 0),
                        stop=(j == npt - 1),
                        skip_group_check=True,
                    )

            steps = []
            for j in range(npt):
                steps.append(lambda j=j: (scores(j), j > 1 and rowsums(j - 2)))
            steps.append(lambda: rowsums(npt - 2))
            steps.append(lambda: rowsums(npt - 1))
            return steps, shared

        def stream_B(t, f, shared, attn_t):
            """Reciprocal + broadcast + attn@V for the unit A just finished.

            V matmuls lag the bt/pt-mul macro by one so the PE never waits on
            the DVE/GPSIMD probability multiplies.  pt-muls split h0->DVE,
            h1->GPSIMD.
            """
            npt = NP_T[t]
            lo = LO[t]
            mB = npt // 2
            st = {}

            def recip():
                rc = rcpp.tile([64, 512], BF16, name="rc", tag="rcp")
                rs1 = rcpp.tile([64, 512], F32, name="rs1", tag="rcs1")
                rs2 = rcpp.tile([64, 512], F32, name="rs2", tag="rcs2")
                nc.vector.reciprocal_approx_accurate(rs2[:], shared["accs"][0:64, :], rs1[:])
                nc.vector.tensor_mul(rc[:], rs2[:], vm[:, t * QS : (t + 1) * QS])
                st["rc"] = rc

            def btmul(j):
                rc = st["rc"]
                pts = []
                for h in range(2):
                    hb = 32 * h
                    expS = shared["expS0" if h == 0 else "expS1"]
                    bt = btp.tile([128, 512], F32, name="bt", tag="btP")
                    nc.tensor.matmul(
                        bt[:],
                        bds[hb : hb + 2 * npt, j * 128 : (j + 1) * 128],
                        rc[hb : hb + 2 * npt, :],
                        start=True,
                        stop=True,
                    )
                    ptt = ptp.tile([128, 512], BF16, name="ptt", tag="pt")
                    nc.vector.tensor_mul(ptt[:], expS[:, j * QS : (j + 1) * QS], bt[:])
                    pts.append(ptt)
                st[("pt", j)] = pts

            def vmm(j):
                if j == 0:
                    st["acco"] = accop.tile([128, 512], F32, name="acco", tag="accoP")
                acco = st["acco"]
                pts = st.pop(("pt", j))
                cp = lo // 2 + j
                for h in range(2):
                    nc.tensor.matmul(
                        acco[64 * h : 64 * h + 64, :],
                        vv[:, cp * F + (2 * f + h) * 64 : cp * F + (2 * f + h) * 64 + 64],
                        pts[h][:],
                        start=(j == 0),
                        stop=(j == npt - 1),
                        skip_group_check=True,
                    )

            steps = [recip]
            for j in range(npt):
                steps.append(lambda j=j: (btmul(j), j > 1 and vmm(j - 2)))
            steps.append(lambda: vmm(npt - 2))
            steps.append(lambda: (vmm(npt - 1), nc.scalar.copy(attn_t[:], st["acco"][:])))
            return steps

        def stream_C(t, atts):
            """Output projection of slab t, dripped 2 matmuls per macro step."""
            st = {}

            def piece(sc2, eh):
                if eh == 0:
                    st["ob"] = outp.tile([128, 1024], F32, name="ob", tag="outsb")
                po = btp.tile([128, 512], F32, name="po", tag="btP")
                for f in range(2):
                    nc.tensor.matmul(
                        po[:],
                        atts[f][:, sc2 * 128 : sc2 * 128 + 128],
                        wo[:, f * E + eh * 512 : f * E + eh * 512 + 512],
                        start=(f == 0),
                        stop=(f == 1),
                    )
                nc.vector.tensor_copy(st["ob"][:, eh * 512 : (eh + 1) * 512], po[:])
                if eh == 1:
                    row = (4 * t + sc2) * 128
                    nc.sync.dma_start(out_d.ap()[row : row + 128, :], st["ob"][:])

            steps = []
            for sc2 in range(4):
                for eh in range(2):
                    steps.append(lambda sc2=sc2, eh=eh: piece(sc2, eh))
                    steps.append(lambda: None)  # drip at half rate: po shares btP slots
            return steps

        def merge(streams):
            for k in range(max(len(s) for s in streams)):
                for s in streams:
                    if k < len(s):
                        s[k]()

        units = [(t, f) for t in range(T_SLABS) for f in range(2)]
        atts_by_t = {t: [] for t in range(T_SLABS)}
        pending_B = None
        for i, (t, f) in enumerate(units):
            sA, shared = stream_A(t, f)
            streams = [sA]
            if pending_B is not None:
                streams.append(pending_B)
            if f == 1 and t >= 1:
                streams.append(stream_C(t - 1, atts_by_t[t - 1]))
            merge(streams)
            attn_t = attp.tile([128, 512], BF16, name="attn_t", tag="att")
            atts_by_t[t].append(attn_t)
            pending_B = stream_B(t, f, shared, attn_t)
        merge([pending_B])
        merge([stream_C(T_SLABS - 1, atts_by_t[T_SLABS - 1])])

    nc.compile()
    return nc


_NC_CACHE = []


def _get_nc():
    if not _NC_CACHE:
        _NC_CACHE.append(build_nc())
    return _NC_CACHE[0]


def _host_consts():
    selc = np.zeros((128, MAXP * 32), np.float32)
    for k in range(128):
        for j in range(MAXP):
            selc[k, j * 32 + 2 * j + k // 64] = 1.0
    bdsel = np.zeros((64, MAXP * 128), np.float32)
    for j in range(MAXP):
        for p in range(128):
            bdsel[2 * j + p // 64, j * 128 + p] = 1.0
            bdsel[32 + 2 * j + p // 64, j * 128 + p] = 1.0
    vmask = np.zeros((64, T_SLABS * QS), np.float32)
    for t in range(T_SLABS):
        for m in range(2 * NP_T[t]):
            c = LO[t] + m
            for qb in range(QS // BLK):
                r = 8 * t + qb
                if abs(r - c) <= BAND:
                    vmask[m, t * QS + qb * 64 : t * QS + (qb + 1) * 64] = 1.0
                    vmask[32 + m, t * QS + qb * 64 : t * QS + (qb + 1) * 64] = 1.0
    return (
        selc.astype(BFD),
        bdsel.astype(BFD),
        vmask.astype(BFD),
    )


def build_in_maps(query, key, value, Wq, Wk, Wv, Wo):
    query = np.asarray(query, np.float32)
    key = np.asarray(key, np.float32)
    value = np.asarray(value, np.float32)
    Wq = np.asarray(Wq, np.float32)
    Wk = np.asarray(Wk, np.float32)
    Wv = np.asarray(Wv, np.float32)
    Wo = np.asarray(Wo, np.float32)

    selc, bdsel, vmask = _host_consts()
    xs = [np.ascontiguousarray(a[b].T).astype(BFD) for a in (query, key, value) for b in range(B)]
    in_maps = []
    for c in range(NCORES):
        b, g = divmod(c, HPC)
        fs = slice(F * g, F * (g + 1))
        in_maps.append(
            {
                "xqT": xs[0 + b],
                "xkT": xs[2 + b],
                "xvT": xs[4 + b],
                "wqT": np.ascontiguousarray((Wq[fs, :] * SCALE).T).astype(BFD),
                "wkT": np.ascontiguousarray(Wk[fs, :].T).astype(BFD),
                "wvT": np.ascontiguousarray(Wv[fs, :].T).astype(BFD),
                "woT": np.ascontiguousarray(Wo[:, fs].T).astype(BFD),
                "selc": selc,
                "bdsel": bdsel,
                "vmask": vmask,
            }
        )
    return in_maps


def kernel(query, key, value, Wq, Wk, Wv, Wo):
    nc = _get_nc()
    in_maps = build_in_maps(query, key, value, Wq, Wk, Wv, Wo)
    res = bass_utils.run_bass_kernel_spmd(nc, in_maps, core_ids=list(range(NCORES)))
    out = np.zeros((B, S, E), np.float32)
    for c in range(NCORES):
        b = c // HPC
        out[b] += res.results[c]["out"]
    return out


# revision 22
# speedup vs baseline: 1.6356x; 1.0113x over previous
"""Block-sparse (banded) attention kernel for Trainium2, 8 NeuronCores.

Sharding: data-parallel over batch (2) x tensor-parallel over heads
(16 heads -> 4 per core).  Each core computes its 4 heads' Q/K/V
projections, banded block attention (|r-c| <= 15 blocks, per-block
softmax), and a partial output projection; the host sums the 4 partial
outputs per batch element.

All matmul operands are bf16 (PSUM accumulation stays fp32).  Heads are
processed in pairs per fold; scores / row-sum / broadcast / attn@V
matmuls run as concurrent tile_position'd pairs.  Phase 3 merges the
scores pipeline of unit u with the value pipeline of unit u-1 and the
output projection of the previous slab at macro-step granularity so the
in-order PE queue never chains behind ACT/DVE latency.

Self-contained: hardcodes all shapes; only needs the concourse tree that
the environment already puts on sys.path.
"""

import sys

for _p in ("/opt/trn_rl_repo",):
    if _p not in sys.path:
        sys.path.insert(0, _p)

from contextlib import ExitStack

import numpy as np
import ml_dtypes

import concourse.bacc as bacc
import concourse.tile as tile
from concourse import bass_utils, mybir

F32 = mybir.dt.float32
BF16 = mybir.dt.bfloat16
EXP = mybir.ActivationFunctionType.Exp

B, S, E = 2, 2048, 1024
H, HD, BLK = 16, 64, 64
NB = S // BLK  # 32 blocks
NCORES = 8
HPC = 4  # heads per core
F = HPC * HD  # 256 local features
BAND = 15
SCALE = HD ** -0.5
BFD = ml_dtypes.bfloat16

# per r8-slab (8 query blocks, q=512) column-block ranges, even-extended
T_SLABS = 4
QS = 512  # q extent per slab
LO = []
NP_T = []
for _t in range(T_SLABS):
    lo = max(0, 8 * _t - BAND)
    hi = min(NB - 1, 8 * _t + 7 + BAND)
    if (hi - lo + 1) % 2 == 1:
        if lo > 0:
            lo -= 1
        else:
            hi += 1
    LO.append(lo)
    NP_T.append((hi - lo + 1) // 2)
MAXP = max(NP_T)  # 16 pairs


def build_nc():
    nc = bacc.Bacc("TRN2", target_bir_lowering=False, debug=False)

    xq_d = nc.dram_tensor("xqT", [E, S], BF16, kind="ExternalInput")
    xk_d = nc.dram_tensor("xkT", [E, S], BF16, kind="ExternalInput")
    xv_d = nc.dram_tensor("xvT", [E, S], BF16, kind="ExternalInput")
    wq_d = nc.dram_tensor("wqT", [E, F], BF16, kind="ExternalInput")
    wk_d = nc.dram_tensor("wkT", [E, F], BF16, kind="ExternalInput")
    wv_d = nc.dram_tensor("wvT", [E, F], BF16, kind="ExternalInput")
    wo_d = nc.dram_tensor("woT", [F, E], BF16, kind="ExternalInput")
    sel_d = nc.dram_tensor("selc", [128, MAXP * 32], BF16, kind="ExternalInput")
    bds_d = nc.dram_tensor("bdsel", [64, MAXP * 128], BF16, kind="ExternalInput")
    vm_d = nc.dram_tensor("vmask", [64, T_SLABS * QS], BF16, kind="ExternalInput")
    out_d = nc.dram_tensor("out", [S, E], F32, kind="ExternalOutput")

    with tile.TileContext(nc) as tc, ExitStack() as ctx, nc.allow_low_precision(
        reason="bf16 pipeline; fp32 PSUM accumulation throughout"
    ):
        pers = ctx.enter_context(tc.tile_pool(name="pers", bufs=1))
        qT = pers.tile([128, 2 * S], BF16, tag="qT")
        kT = pers.tile([128, 2 * S], BF16, tag="kT")
        vv = pers.tile([128, 16 * F], BF16, tag="vv")
        wq = pers.tile([128, 8 * F], BF16, tag="wq")
        wk = pers.tile([128, 8 * F], BF16, tag="wk")
        wv = pers.tile([128, 8 * F], BF16, tag="wv")
        wo = pers.tile([128, 2 * E], BF16, tag="wo")
        selb = pers.tile([128, MAXP * 32], BF16, tag="selb")
        bds = pers.tile([64, MAXP * 128], BF16, tag="bds")
        vm = pers.tile([64, T_SLABS * QS], BF16, tag="vm")

        # k-projection weights first: phase 1 is on the critical path
        nc.sync.dma_start(
            wk[:].rearrange("p (c f) -> p c f", c=8),
            wk_d.ap().rearrange("(c p) f -> p c f", p=128),
        )
        # remaining weights/constants arrive via gpsimd (SWDGE) so they don't
        # queue ahead of the phase-1/2 x-tile loads on the sync ring
        nc.gpsimd.dma_start(
            wv[:].rearrange("p (c f) -> p c f", c=8),
            wv_d.ap().rearrange("(c p) f -> p c f", p=128),
        )
        nc.gpsimd.dma_start(
            wq[:].rearrange("p (c f) -> p c f", c=8),
            wq_d.ap().rearrange("(c p) f -> p c f", p=128),
        )
        nc.gpsimd.dma_start(
            wo[:].rearrange("p (c e) -> p c e", c=2),
            wo_d.ap().rearrange("(c p) e -> p c e", p=128),
        )
        nc.gpsimd.dma_start(selb[:], sel_d.ap())
        nc.gpsimd.dma_start(bds[:], bds_d.ap())
        nc.gpsimd.dma_start(vm[:], vm_d.ap())

        # big staging tiles for xv/xq: loaded as full [128, 2048] rows (4KB
        # per-partition DMA lines) while phase 1 computes; freed before the
        # attention pools open
        xbig = tc.tile_pool(name="xbig", bufs=1)
        xbp = xbig.__enter__()
        xvAll = xbp.tile([128, 8 * S], BF16, tag="xvAll")
        xqAll = xbp.tile([128, 8 * S], BF16, tag="xqAll")

        # ---- phase 1: k projection (kT layout [f, s]) ----
        with tc.tile_pool(name="xk", bufs=2) as xkp, tc.tile_pool(
            name="psK", bufs=1, space="PSUM"
        ) as pskp:
            psK = pskp.tile([128, 4096], F32)
            for e in range(8):
                xt = xkp.tile([128, S], BF16, tag="xk")
                nc.sync.dma_start(xt[:], xk_d.ap()[e * 128 : (e + 1) * 128, :])
                nc.sync.dma_start(
                    xvAll[:, e * S : (e + 1) * S], xv_d.ap()[e * 128 : (e + 1) * 128, :]
                )
                nc.sync.dma_start(
                    xqAll[:, e * S : (e + 1) * S], xq_d.ap()[e * 128 : (e + 1) * 128, :]
                )
                for fold in range(2):
                    for sc in range(4):
                        nc.tensor.matmul(
                            psK[:, (fold * 4 + sc) * 512 : (fold * 4 + sc + 1) * 512],
                            wk[:, e * F + fold * 128 : e * F + fold * 128 + 128],
                            xt[:, sc * 512 : (sc + 1) * 512],
                            start=(e == 0),
                            stop=(e == 7),
                        )
            for fold in range(2):
                for sc in range(4):
                    nc.scalar.copy(
                        kT[:, fold * S + sc * 512 : fold * S + (sc + 1) * 512],
                        psK[:, (fold * 4 + sc) * 512 : (fold * 4 + sc + 1) * 512],
                    )

        # ---- phase 2: v projection (natural layout [s, f]) ----
        # x already staged in xvAll; one accumulation group per PSUM bank
        with tc.tile_pool(name="psV", bufs=2, space="PSUM") as psvp:
            for sc in range(4):
                pvs = [
                    psvp.tile([128, 256], F32, name=f"pv{sub}", tag=f"psV{sub}")
                    for sub in range(4)
                ]
                for e in range(8):
                    for sub in range(4):
                        nc.tensor.matmul(
                            pvs[sub][:],
                            xvAll[:, e * S + sc * 512 + sub * 128 : e * S + sc * 512 + (sub + 1) * 128],
                            wv[:, e * F : (e + 1) * F],
                            start=(e == 0),
                            stop=(e == 7),
                        )
                for sub in range(4):
                    nc.scalar.copy(
                        vv[:, sc * 1024 + sub * 256 : sc * 1024 + (sub + 1) * 256],
                        pvs[sub][:],
                    )

        # ---- phase 2.5: q projection for all four slabs ----
        with tc.tile_pool(name="psQ", bufs=2, space="PSUM") as psqp:
            for sc4 in range(4):
                pqs = [
                    psqp.tile([128, 512], F32, name=f"pq{fold}", tag=f"psQ{fold}")
                    for fold in range(2)
                ]
                for e in range(8):
                    for fold in range(2):
                        nc.tensor.matmul(
                            pqs[fold][:],
                            wq[:, e * F + fold * 128 : e * F + fold * 128 + 128],
                            xqAll[:, e * S + sc4 * 512 : e * S + (sc4 + 1) * 512],
                            start=(e == 0),
                            stop=(e == 7),
                        )
                for fold in range(2):
                    nc.scalar.copy(
                        qT[:, fold * S + sc4 * 512 : fold * S + (sc4 + 1) * 512],
                        pqs[fold][:],
                    )
        xbig.__exit__(None, None, None)

        # ---- phase 3: merged attention pipeline + output projection ----
        # PSUM budget (8 banks): psH [128,512]x4 = 4, bt/po [128,512]x2 = 2,
        # accs x1 = 1, acco x1 = 1.  All cross-engine consumers lag their
        # producers by two j-steps so the PE issue stream never carries an
        # unsatisfied wait (keeps HAM at K=8/8).
        psHp = ctx.enter_context(tc.tile_pool(name="psH", bufs=4, space="PSUM"))
        btp = ctx.enter_context(tc.tile_pool(name="btP", bufs=2, space="PSUM"))
        accp = ctx.enter_context(tc.tile_pool(name="accP", bufs=1, space="PSUM"))
        accop = ctx.enter_context(tc.tile_pool(name="accoP", bufs=1, space="PSUM"))
        expp = ctx.enter_context(tc.tile_pool(name="expS", bufs=4))
        ptp = ctx.enter_context(tc.tile_pool(name="pt", bufs=8))
        rcpp = ctx.enter_context(tc.tile_pool(name="rcp", bufs=2))
        attp = ctx.enter_context(tc.tile_pool(name="att", bufs=4))
        outp = ctx.enter_context(tc.tile_pool(name="outsb", bufs=2))

        def stream_A(t, f):
            """Scores + exp + row sums for head pair (2f, 2f+1) of slab t.

            Returns (steps, shared) where shared collects the tiles stream_B
            needs.  Macro step k covers j pair (2k, 2k+1); row sums lag one
            macro so they never chain the PE behind the ACT exp latency.
            """
            npt = NP_T[t]
            lo = LO[t]
            mA = npt // 2
            shared = {}

            def alloc(_k=0):
                shared["expS0"] = expp.tile([128, MAXP * QS], BF16, name="expS0", tag="expS")
                shared["expS1"] = expp.tile([128, MAXP * QS], BF16, name="expS1", tag="expS")

            def scores(j):
                if j == 0:
                    alloc()
                c0 = lo + 2 * j
                for h, base in ((0, 0), (1, 64)):
                    ps = psHp.tile([128, 512], F32, name="psH", tag="psH")
                    nc.tensor.matmul(
                        ps[:],
                        kT[base : base + 64, f * S + c0 * 64 : f * S + c0 * 64 + 128],
                        qT[base : base + 64, f * S + t * QS : f * S + (t + 1) * QS],
                        start=True,
                        stop=True,
                    )
                    nc.scalar.activation(
                        shared["expS0" if h == 0 else "expS1"][:, j * QS : (j + 1) * QS],
                        ps[:],
                        EXP,
                    )

            def rowsums(j):
                if j == 0:
                    shared["accs"] = accp.tile([128, 512], F32, name="accs", tag="accP")
                accs = shared["accs"]
                for h, expS in ((0, shared["expS0"]), (1, shared["expS1"])):
                    nc.tensor.matmul(
                        accs[32 * h : 32 * h + 32, :],
                        selb[:, j * 32 : (j + 1) * 32],
                        expS[:, j * QS : (j + 1) * QS],
                        start=(j == 0),
                        stop=(j == npt - 1),
                        skip_group_check=True,
                    )

            steps = []
            for j in range(npt):
                steps.append(lambda j=j: (scores(j), j > 1 and rowsums(j - 2)))
            steps.append(lambda: rowsums(npt - 2))
            steps.append(lambda: rowsums(npt - 1))
            return steps, shared

        def stream_B(t, f, shared, attn_t):
            """Reciprocal + broadcast + attn@V for the unit A just finished.

            V matmuls lag the bt/pt-mul macro by one so the PE never waits on
            the DVE/GPSIMD probability multiplies.  pt-muls split h0->DVE,
            h1->GPSIMD.
            """
            npt = NP_T[t]
            lo = LO[t]
            mB = npt // 2
            st = {}

            def recip():
                rc = rcpp.tile([64, 512], BF16, name="rc", tag="rcp")
                rs1 = rcpp.tile([64, 512], F32, name="rs1", tag="rcs1")
                rs2 = rcpp.tile([64, 512], F32, name="rs2", tag="rcs2")
                nc.vector.reciprocal_approx_accurate(rs2[:], shared["accs"][0:64, :], rs1[:])
                nc.vector.tensor_mul(rc[:], rs2[:], vm[:, t * QS : (t + 1) * QS])
                st["rc"] = rc

            def btmul(j):
                rc = st["rc"]
                pts = []
                for h in range(2):
                    hb = 32 * h
                    expS = shared["expS0" if h == 0 else "expS1"]
                    bt = btp.tile([128, 512], F32, name="bt", tag="btP")
                    nc.tensor.matmul(
                        bt[:],
                        bds[hb : hb + 2 * npt, j * 128 : (j + 1) * 128],
                        rc[hb : hb + 2 * npt, :],
                        start=True,
                        stop=True,
                    )
                    ptt = ptp.tile([128, 512], BF16, name="ptt", tag="pt")
                    nc.vector.tensor_mul(ptt[:], expS[:, j * QS : (j + 1) * QS], bt[:])
                    pts.append(ptt)
                st[("pt", j)] = pts

            def vmm(j):
                if j == 0:
                    st["acco"] = accop.tile([128, 512], F32, name="acco", tag="accoP")
                acco = st["acco"]
                pts = st.pop(("pt", j))
                cp = lo // 2 + j
                for h in range(2):
                    nc.tensor.matmul(
                        acco[64 * h : 64 * h + 64, :],
                        vv[:, cp * F + (2 * f + h) * 64 : cp * F + (2 * f + h) * 64 + 64],
                        pts[h][:],
                        start=(j == 0),
                        stop=(j == npt - 1),
                        skip_group_check=True,
                    )

            steps = [recip]
            for j in range(npt):
                steps.append(lambda j=j: (btmul(j), j > 1 and vmm(j - 2)))
            steps.append(lambda: vmm(npt - 2))
            steps.append(lambda: (vmm(npt - 1), nc.scalar.copy(attn_t[:], st["acco"][:])))
            return steps

        def stream_C(t, atts, pad=True):
            """Output projection of slab t, dripped 2 matmuls per macro step."""
            st = {}

            def piece(sc2, eh):
                if eh == 0:
                    st["ob"] = outp.tile([128, 1024], F32, name="ob", tag="outsb")
                po = btp.tile([128, 512], F32, name="po", tag="btP")
                for f in range(2):
                    nc.tensor.matmul(
                        po[:],
                        atts[f][:, sc2 * 128 : sc2 * 128 + 128],
                        wo[:, f * E + eh * 512 : f * E + eh * 512 + 512],
                        start=(f == 0),
                        stop=(f == 1),
                    )
                nc.vector.tensor_copy(st["ob"][:, eh * 512 : (eh + 1) * 512], po[:])
                if eh == 1:
                    row = (4 * t + sc2) * 128
                    nc.sync.dma_start(out_d.ap()[row : row + 128, :], st["ob"][:])

            steps = []
            for sc2 in range(4):
                for eh in range(2):
                    steps.append(lambda sc2=sc2, eh=eh: piece(sc2, eh))
                    if pad:
                        steps.append(lambda: None)  # half rate: po shares btP slots
            return steps

        def merge(streams):
            for k in range(max(len(s) for s in streams)):
                for s in streams:
                    if k < len(s):
                        s[k]()

        units = [(t, f) for t in range(T_SLABS) for f in range(2)]
        atts_by_t = {t: [] for t in range(T_SLABS)}
        pending_B = None
        for i, (t, f) in enumerate(units):
            sA, shared = stream_A(t, f)
            streams = [sA]
            if pending_B is not None:
                streams.append(pending_B)
            if f == 1 and t >= 1:
                streams.append(stream_C(t - 1, atts_by_t[t - 1]))
            merge(streams)
            attn_t = attp.tile([128, 512], BF16, name="attn_t", tag="att")
            atts_by_t[t].append(attn_t)
            pending_B = stream_B(t, f, shared, attn_t)
        merge([pending_B])
        merge([stream_C(T_SLABS - 1, atts_by_t[T_SLABS - 1], pad=False)])

    nc.compile()
    return nc


_NC_CACHE = []


def _get_nc():
    if not _NC_CACHE:
        _NC_CACHE.append(build_nc())
    return _NC_CACHE[0]


def _host_consts():
    selc = np.zeros((128, MAXP * 32), np.float32)
    for k in range(128):
        for j in range(MAXP):
            selc[k, j * 32 + 2 * j + k // 64] = 1.0
    bdsel = np.zeros((64, MAXP * 128), np.float32)
    for j in range(MAXP):
        for p in range(128):
            bdsel[2 * j + p // 64, j * 128 + p] = 1.0
            bdsel[32 + 2 * j + p // 64, j * 128 + p] = 1.0
    vmask = np.zeros((64, T_SLABS * QS), np.float32)
    for t in range(T_SLABS):
        for m in range(2 * NP_T[t]):
            c = LO[t] + m
            for qb in range(QS // BLK):
                r = 8 * t + qb
                if abs(r - c) <= BAND:
                    vmask[m, t * QS + qb * 64 : t * QS + (qb + 1) * 64] = 1.0
                    vmask[32 + m, t * QS + qb * 64 : t * QS + (qb + 1) * 64] = 1.0
    return (
        selc.astype(BFD),
        bdsel.astype(BFD),
        vmask.astype(BFD),
    )


def build_in_maps(query, key, value, Wq, Wk, Wv, Wo):
    query = np.asarray(query, np.float32)
    key = np.asarray(key, np.float32)
    value = np.asarray(value, np.float32)
    Wq = np.asarray(Wq, np.float32)
    Wk = np.asarray(Wk, np.float32)
    Wv = np.asarray(Wv, np.float32)
    Wo = np.asarray(Wo, np.float32)

    selc, bdsel, vmask = _host_consts()
    xs = [np.ascontiguousarray(a[b].T).astype(BFD) for a in (query, key, value) for b in range(B)]
    in_maps = []
    for c in range(NCORES):
        b, g = divmod(c, HPC)
        fs = slice(F * g, F * (g + 1))
        in_maps.append(
            {
                "xqT": xs[0 + b],
                "xkT": xs[2 + b],
                "xvT": xs[4 + b],
                "wqT": np.ascontiguousarray((Wq[fs, :] * SCALE).T).astype(BFD),
                "wkT": np.ascontiguousarray(Wk[fs, :].T).astype(BFD),
                "wvT": np.ascontiguousarray(Wv[fs, :].T).astype(BFD),
                "woT": np.ascontiguousarray(Wo[:, fs].T).astype(BFD),
                "selc": selc,
                "bdsel": bdsel,
                "vmask": vmask,
            }
        )
    return in_maps


def kernel(query, key, value, Wq, Wk, Wv, Wo):
    nc = _get_nc()
    in_maps = build_in_maps(query, key, value, Wq, Wk, Wv, Wo)
    res = bass_utils.run_bass_kernel_spmd(nc, in_maps, core_ids=list(range(NCORES)))
    out = np.zeros((B, S, E), np.float32)
    for c in range(NCORES):
        b = c // HPC
        out[b] += res.results[c]["out"]
    return out


# revision 33
# speedup vs baseline: 1.7697x; 1.0820x over previous
"""Block-sparse (banded) attention kernel for Trainium2, 8 NeuronCores.

Sharding: data-parallel over batch (2) x tensor-parallel over heads
(16 heads -> 4 per core).  Each core computes its 4 heads' Q/K/V
projections, banded block attention (|r-c| <= 15 blocks, per-block
softmax), and a partial output projection; the host sums the 4 partial
outputs per batch element.

All matmul operands are bf16 (PSUM accumulation stays fp32).  Heads are
processed in pairs per fold; scores / row-sum / broadcast / attn@V
matmuls run as concurrent tile_position'd pairs.  Phase 3 merges the
scores pipeline of unit u with the value pipeline of unit u-1 and the
output projection of the previous slab at macro-step granularity so the
in-order PE queue never chains behind ACT/DVE latency.

Self-contained: hardcodes all shapes; only needs the concourse tree that
the environment already puts on sys.path.
"""

import sys

for _p in ("/opt/trn_rl_repo",):
    if _p not in sys.path:
        sys.path.insert(0, _p)

from contextlib import ExitStack

import numpy as np
import ml_dtypes

import concourse.bacc as bacc
import concourse.tile as tile
from concourse import bass_utils, mybir

F32 = mybir.dt.float32
BF16 = mybir.dt.bfloat16
EXP = mybir.ActivationFunctionType.Exp

B, S, E = 2, 2048, 1024
H, HD, BLK = 16, 64, 64
NB = S // BLK  # 32 blocks
NCORES = 8
HPC = 4  # heads per core
F = HPC * HD  # 256 local features
BAND = 15
SCALE = HD ** -0.5
BFD = ml_dtypes.bfloat16

# per r8-slab (8 query blocks, q=512) column-block ranges, even-extended
T_SLABS = 4
QS = 512  # q extent per slab
LO = []
NP_T = []
for _t in range(T_SLABS):
    lo = max(0, 8 * _t - BAND)
    hi = min(NB - 1, 8 * _t + 7 + BAND)
    if (hi - lo + 1) % 2 == 1:
        if lo > 0:
            lo -= 1
        else:
            hi += 1
    LO.append(lo)
    NP_T.append((hi - lo + 1) // 2)
MAXP = max(NP_T)  # 16 pairs


def build_nc():
    nc = bacc.Bacc("TRN2", target_bir_lowering=False, debug=False)

    xq_d = nc.dram_tensor("xqT", [E, S], BF16, kind="ExternalInput")
    xk_d = nc.dram_tensor("xkT", [E, S], BF16, kind="ExternalInput")
    xv_d = nc.dram_tensor("xvT", [E, S], BF16, kind="ExternalInput")
    wq_d = nc.dram_tensor("wqT", [E, F], BF16, kind="ExternalInput")
    wk_d = nc.dram_tensor("wkT", [E, F], BF16, kind="ExternalInput")
    wv_d = nc.dram_tensor("wvT", [E, F], BF16, kind="ExternalInput")
    wo_d = nc.dram_tensor("woT", [F, E], BF16, kind="ExternalInput")
    sel_d = nc.dram_tensor("selc", [128, MAXP * 32], BF16, kind="ExternalInput")
    vm_d = nc.dram_tensor("vmask", [64, T_SLABS * QS], BF16, kind="ExternalInput")
    out_d = nc.dram_tensor("out", [S, E], F32, kind="ExternalOutput")
    # ping-pong DRAM staging for the reciprocal broadcast (DMA partition
    # replication needs a DRAM source: SBUF APs can't have zero-step
    # partition dims)
    scr_d = [
        nc.dram_tensor(f"rcscr{i}", [64, 512], BF16, kind="Internal") for i in range(2)
    ]

    with tile.TileContext(nc) as tc, ExitStack() as ctx, nc.allow_low_precision(
        reason="bf16 pipeline; fp32 PSUM accumulation throughout"
    ):
        pers = ctx.enter_context(tc.tile_pool(name="pers", bufs=1))
        qT = pers.tile([128, 2 * S], BF16, tag="qT")
        kT = pers.tile([128, 2 * S], BF16, tag="kT")
        vv = pers.tile([128, 16 * F], BF16, tag="vv")
        wq = pers.tile([128, 8 * F], BF16, tag="wq")
        wk = pers.tile([128, 8 * F], BF16, tag="wk")
        wv = pers.tile([128, 8 * F], BF16, tag="wv")
        wo = pers.tile([128, 2 * E], BF16, tag="wo")
        selb = pers.tile([128, MAXP * 32], BF16, tag="selb")
        vm = pers.tile([64, T_SLABS * QS], BF16, tag="vm")

        # k-projection weights first: phase 1 is on the critical path
        nc.sync.dma_start(
            wk[:].rearrange("p (c f) -> p c f", c=8),
            wk_d.ap().rearrange("(c p) f -> p c f", p=128),
        )
        # remaining weights/constants arrive via gpsimd (SWDGE) so they don't
        # queue ahead of the phase-1/2 x-tile loads on the sync ring
        nc.gpsimd.dma_start(
            wv[:].rearrange("p (c f) -> p c f", c=8),
            wv_d.ap().rearrange("(c p) f -> p c f", p=128),
        )
        nc.gpsimd.dma_start(
            wq[:].rearrange("p (c f) -> p c f", c=8),
            wq_d.ap().rearrange("(c p) f -> p c f", p=128),
        )
        nc.gpsimd.dma_start(
            wo[:].rearrange("p (c e) -> p c e", c=2),
            wo_d.ap().rearrange("(c p) e -> p c e", p=128),
        )
        nc.gpsimd.dma_start(selb[:], sel_d.ap())
        nc.gpsimd.dma_start(vm[:], vm_d.ap())

        # big staging tiles for xv/xq: loaded as full [128, 2048] rows (4KB
        # per-partition DMA lines) while phase 1 computes; freed before the
        # attention pools open
        xbig = tc.tile_pool(name="xbig", bufs=1)
        xbp = xbig.__enter__()
        xvAll = xbp.tile([128, 8 * S], BF16, tag="xvAll")
        xqAll = xbp.tile([128, 8 * S], BF16, tag="xqAll")

        # ---- phase 1: k projection (kT layout [f, s]) ----
        with tc.tile_pool(name="xk", bufs=2) as xkp, tc.tile_pool(
            name="psK", bufs=1, space="PSUM"
        ) as pskp:
            psK = pskp.tile([128, 4096], F32)
            for e in range(8):
                xt = xkp.tile([128, S], BF16, tag="xk")
                nc.sync.dma_start(xt[:], xk_d.ap()[e * 128 : (e + 1) * 128, :])
                nc.sync.dma_start(
                    xvAll[:, e * S : (e + 1) * S], xv_d.ap()[e * 128 : (e + 1) * 128, :]
                )
                nc.sync.dma_start(
                    xqAll[:, e * S : (e + 1) * S], xq_d.ap()[e * 128 : (e + 1) * 128, :]
                )
                for fold in range(2):
                    for sc in range(4):
                        nc.tensor.matmul(
                            psK[:, (fold * 4 + sc) * 512 : (fold * 4 + sc + 1) * 512],
                            wk[:, e * F + fold * 128 : e * F + fold * 128 + 128],
                            xt[:, sc * 512 : (sc + 1) * 512],
                            start=(e == 0),
                            stop=(e == 7),
                        )
            for fold in range(2):
                for sc in range(4):
                    nc.scalar.copy(
                        kT[:, fold * S + sc * 512 : fold * S + (sc + 1) * 512],
                        psK[:, (fold * 4 + sc) * 512 : (fold * 4 + sc + 1) * 512],
                    )

        # ---- phase 2: v projection (natural layout [s, f]) ----
        # x already staged in xvAll; one accumulation group per PSUM bank
        with tc.tile_pool(name="psV", bufs=2, space="PSUM") as psvp:
            for sc in range(4):
                pvs = [
                    psvp.tile([128, 256], F32, name=f"pv{sub}", tag=f"psV{sub}")
                    for sub in range(4)
                ]
                for e in range(8):
                    for sub in range(4):
                        nc.tensor.matmul(
                            pvs[sub][:],
                            xvAll[:, e * S + sc * 512 + sub * 128 : e * S + sc * 512 + (sub + 1) * 128],
                            wv[:, e * F : (e + 1) * F],
                            start=(e == 0),
                            stop=(e == 7),
                        )
                for sub in range(4):
                    nc.scalar.copy(
                        vv[:, sc * 1024 + sub * 256 : sc * 1024 + (sub + 1) * 256],
                        pvs[sub][:],
                    )

        # ---- phase 2.5: q projection for all four slabs ----
        with tc.tile_pool(name="psQ", bufs=2, space="PSUM") as psqp:
            for sc4 in range(4):
                pqs = [
                    psqp.tile([128, 512], F32, name=f"pq{fold}", tag=f"psQ{fold}")
                    for fold in range(2)
                ]
                for e in range(8):
                    for fold in range(2):
                        nc.tensor.matmul(
                            pqs[fold][:],
                            wq[:, e * F + fold * 128 : e * F + fold * 128 + 128],
                            xqAll[:, e * S + sc4 * 512 : e * S + (sc4 + 1) * 512],
                            start=(e == 0),
                            stop=(e == 7),
                        )
                for fold in range(2):
                    nc.scalar.copy(
                        qT[:, fold * S + sc4 * 512 : fold * S + (sc4 + 1) * 512],
                        pqs[fold][:],
                    )
        xbig.__exit__(None, None, None)

        # ---- phase 3: merged attention pipeline + output projection ----
        # PSUM budget (8 banks): psH [128,512]x5 = 5, po x1 = 1, accs x1 = 1,
        # acco x1 = 1.  All cross-engine consumers lag their producers by two
        # j-steps so the PE issue stream never carries an unsatisfied wait.
        # The reciprocal broadcast goes through DRAM + replicating DMAs on the
        # gpsimd SWDGE queue (FIFO-ordered), so the probability multiplies are
        # all-bf16 SBUF ops (DVE 2x mode) and the PE sheds the 224 broadcast
        # matmuls.
        psHp = ctx.enter_context(tc.tile_pool(name="psH", bufs=5, space="PSUM"))
        pop = ctx.enter_context(tc.tile_pool(name="poP", bufs=1, space="PSUM"))
        accp = ctx.enter_context(tc.tile_pool(name="accP", bufs=1, space="PSUM"))
        accop = ctx.enter_context(tc.tile_pool(name="accoP", bufs=1, space="PSUM"))
        expp = ctx.enter_context(tc.tile_pool(name="expS", bufs=4))
        btap = ctx.enter_context(tc.tile_pool(name="btA", bufs=2))
        ptp = ctx.enter_context(tc.tile_pool(name="pt", bufs=6))
        rcpp = ctx.enter_context(tc.tile_pool(name="rcp", bufs=2))
        attp = ctx.enter_context(tc.tile_pool(name="att", bufs=4))
        outp = ctx.enter_context(tc.tile_pool(name="outsb", bufs=2))

        def bcast_dma(scr, bta, h, g, j0, nj):
            """Replicate scr rows 32h+2j+g (j in [j0,j0+nj)) across dst
            partition group g of bta, one row per 512-wide j column block."""
            src = (
                scr.ap()[32 * h : 32 * h + 32, :]
                .rearrange("(j g) c -> g j c", g=2)[g][j0 : j0 + nj]
                .unsqueeze(0)
                .to_broadcast([64, nj, 512])
            )
            dst = bta[g * 64 : (g + 1) * 64, j0 * 512 : (j0 + nj) * 512].rearrange(
                "p (j c) -> p j c", c=512
            )
            nc.gpsimd.dma_start(dst, src)

        def stream_A(t, f, ui):
            """Scores + exp + row sums for head pair (2f, 2f+1) of slab t,
            ending with the reciprocal + DRAM bounce + first broadcast DMAs.

            Returns (steps, shared) where shared collects the tiles stream_B
            needs.  Row sums lag scores by one j-step.
            """
            npt = NP_T[t]
            lo = LO[t]
            scr = scr_d[ui % 2]
            shared = {}

            def alloc(_k=0):
                shared["expS0"] = expp.tile([128, MAXP * QS], BF16, name="expS0", tag="expS")
                shared["expS1"] = expp.tile([128, MAXP * QS], BF16, name="expS1", tag="expS")

            def scores(j):
                if j == 0:
                    alloc()
                c0 = lo + 2 * j
                for h, base in ((0, 0), (1, 64)):
                    ps = psHp.tile([128, 512], F32, name="psH", tag="psH")
                    nc.tensor.matmul(
                        ps[:],
                        kT[base : base + 64, f * S + c0 * 64 : f * S + c0 * 64 + 128],
                        qT[base : base + 64, f * S + t * QS : f * S + (t + 1) * QS],
                        start=True,
                        stop=True,
                    )
                    nc.scalar.activation(
                        shared["expS0" if h == 0 else "expS1"][:, j * QS : (j + 1) * QS],
                        ps[:],
                        EXP,
                    )

            def rowsums(j):
                if j == 0:
                    shared["accs"] = accp.tile([128, 512], F32, name="accs", tag="accP")
                accs = shared["accs"]
                for h, expS in ((0, shared["expS0"]), (1, shared["expS1"])):
                    nc.tensor.matmul(
                        accs[32 * h : 32 * h + 32, :],
                        selb[:, j * 32 : (j + 1) * 32],
                        expS[:, j * QS : (j + 1) * QS],
                        start=(j == 0),
                        stop=(j == npt - 1),
                        skip_group_check=True,
                    )

            def recip_bounce():
                rowsums(npt - 1)
                rc = rcpp.tile([64, 512], BF16, name="rc", tag="rcp")
                rs1 = rcpp.tile([64, 512], F32, name="rs1", tag="rcs1", bufs=1)
                rs2 = rcpp.tile([64, 512], F32, name="rs2", tag="rcs2", bufs=1)
                nc.vector.reciprocal_approx_accurate(rs2[:], shared["accs"][0:64, :], rs1[:])
                nc.vector.tensor_mul(rc[:], rs2[:], vm[:, t * QS : (t + 1) * QS])
                nc.gpsimd.dma_start(scr.ap(), rc[:])
                shared["bt"] = [
                    btap.tile([128, MAXP * QS], BF16, name=f"bta{h}", tag=f"btA{h}")
                    for h in range(2)
                ]
                for h in range(2):
                    for g in range(2):
                        bcast_dma(scr, shared["bt"][h], h, g, 0, min(4, npt))

            steps = []
            for j in range(npt):
                steps.append(lambda j=j: (scores(j), j > 0 and rowsums(j - 1)))
            steps.append(recip_bounce)
            return steps, shared

        def stream_B(t, f, ui, shared, attn_t):
            """Probability multiplies + attn@V for the unit A just finished.

            The reciprocal broadcast tensors arrive by DMA (issued in A's tail
            and in early B steps); pt-muls are all-bf16 SBUF DVE ops.  V
            matmuls lag the pt-mul by two j-steps.
            """
            npt = NP_T[t]
            lo = LO[t]
            scr = scr_d[ui % 2]
            st = {}

            def ptmul(j):
                if j == 0:
                    # second broadcast chunk: j 4..npt-1 for all (h, g)
                    for h in range(2):
                        for g in range(2):
                            bcast_dma(scr, shared["bt"][h], h, g, 4, npt - 4)
                pts = []
                for h in range(2):
                    expS = shared["expS0" if h == 0 else "expS1"]
                    ptt = ptp.tile([128, 512], BF16, name="ptt", tag="pt")
                    nc.vector.tensor_mul(
                        ptt[:],
                        expS[:, j * QS : (j + 1) * QS],
                        shared["bt"][h][:, j * QS : (j + 1) * QS],
                    )
                    pts.append(ptt)
                st[("pt", j)] = pts

            def vmm(j):
                if j == 0:
                    st["acco"] = accop.tile([128, 512], F32, name="acco", tag="accoP")
                acco = st["acco"]
                pts = st.pop(("pt", j))
                cp = lo // 2 + j
                for h in range(2):
                    nc.tensor.matmul(
                        acco[64 * h : 64 * h + 64, :],
                        vv[:, cp * F + (2 * f + h) * 64 : cp * F + (2 * f + h) * 64 + 64],
                        pts[h][:],
                        start=(j == 0),
                        stop=(j == npt - 1),
                        skip_group_check=True,
                    )

            steps = []
            for j in range(npt):
                steps.append(lambda j=j: (ptmul(j), j > 1 and vmm(j - 2)))
            steps.append(lambda: vmm(npt - 2))
            steps.append(lambda: (vmm(npt - 1), nc.scalar.copy(attn_t[:], st["acco"][:])))
            return steps

        def stream_C(t, atts, pad=True):
            """Output projection of slab t, dripped 2 matmuls per macro step."""
            st = {}

            def piece(sc2, eh):
                if eh == 0:
                    st["ob"] = outp.tile([128, 1024], F32, name="ob", tag="outsb")
                po = pop.tile([128, 512], F32, name="po", tag="poP")
                for f in range(2):
                    nc.tensor.matmul(
                        po[:],
                        atts[f][:, sc2 * 128 : sc2 * 128 + 128],
                        wo[:, f * E + eh * 512 : f * E + eh * 512 + 512],
                        start=(f == 0),
                        stop=(f == 1),
                    )
                nc.vector.tensor_copy(st["ob"][:, eh * 512 : (eh + 1) * 512], po[:])
                if eh == 1:
                    row = (4 * t + sc2) * 128
                    nc.sync.dma_start(out_d.ap()[row : row + 128, :], st["ob"][:])

            steps = []
            for sc2 in range(4):
                for eh in range(2):
                    steps.append(lambda sc2=sc2, eh=eh: piece(sc2, eh))
                    if pad:
                        steps.append(lambda: None)  # half rate: po shares btP slots
            return steps

        def merge(streams):
            for k in range(max(len(s) for s in streams)):
                for s in streams:
                    if k < len(s):
                        s[k]()

        units = [(t, f) for t in range(T_SLABS) for f in range(2)]
        atts_by_t = {t: [] for t in range(T_SLABS)}
        pending_B = None
        for i, (t, f) in enumerate(units):
            sA, shared = stream_A(t, f, i)
            streams = [sA]
            if pending_B is not None:
                streams.append(pending_B)
            if f == 1 and t >= 1:
                streams.append(stream_C(t - 1, atts_by_t[t - 1], pad=False))
            merge(streams)
            attn_t = attp.tile([128, 512], BF16, name="attn_t", tag="att")
            atts_by_t[t].append(attn_t)
            pending_B = stream_B(t, f, i, shared, attn_t)
        merge([pending_B])
        merge([stream_C(T_SLABS - 1, atts_by_t[T_SLABS - 1], pad=False)])

    nc.compile()
    return nc


_NC_CACHE = []


def _get_nc():
    if not _NC_CACHE:
        _NC_CACHE.append(build_nc())
    return _NC_CACHE[0]


def _host_consts():
    selc = np.zeros((128, MAXP * 32), np.float32)
    for k in range(128):
        for j in range(MAXP):
            selc[k, j * 32 + 2 * j + k // 64] = 1.0
    vmask = np.zeros((64, T_SLABS * QS), np.float32)
    for t in range(T_SLABS):
        for m in range(2 * NP_T[t]):
            c = LO[t] + m
            for qb in range(QS // BLK):
                r = 8 * t + qb
                if abs(r - c) <= BAND:
                    vmask[m, t * QS + qb * 64 : t * QS + (qb + 1) * 64] = 1.0
                    vmask[32 + m, t * QS + qb * 64 : t * QS + (qb + 1) * 64] = 1.0
    return selc.astype(BFD), vmask.astype(BFD)


def build_in_maps(query, key, value, Wq, Wk, Wv, Wo):
    query = np.asarray(query, np.float32)
    key = np.asarray(key, np.float32)
    value = np.asarray(value, np.float32)
    Wq = np.asarray(Wq, np.float32)
    Wk = np.asarray(Wk, np.float32)
    Wv = np.asarray(Wv, np.float32)
    Wo = np.asarray(Wo, np.float32)

    selc, vmask = _host_consts()
    xs = [np.ascontiguousarray(a[b].T).astype(BFD) for a in (query, key, value) for b in range(B)]
    in_maps = []
    for c in range(NCORES):
        b, g = divmod(c, HPC)
        fs = slice(F * g, F * (g + 1))
        in_maps.append(
            {
                "xqT": xs[0 + b],
                "xkT": xs[2 + b],
                "xvT": xs[4 + b],
                "wqT": np.ascontiguousarray((Wq[fs, :] * SCALE).T).astype(BFD),
                "wkT": np.ascontiguousarray(Wk[fs, :].T).astype(BFD),
                "wvT": np.ascontiguousarray(Wv[fs, :].T).astype(BFD),
                "woT": np.ascontiguousarray(Wo[:, fs].T).astype(BFD),
                "selc": selc,
                "vmask": vmask,
            }
        )
    return in_maps


def kernel(query, key, value, Wq, Wk, Wv, Wo):
    nc = _get_nc()
    in_maps = build_in_maps(query, key, value, Wq, Wk, Wv, Wo)
    res = bass_utils.run_bass_kernel_spmd(nc, in_maps, core_ids=list(range(NCORES)))
    out = np.zeros((B, S, E), np.float32)
    for c in range(NCORES):
        b = c // HPC
        out[b] += res.results[c]["out"]
    return out
